# revision 24
# baseline (speedup 1.0000x reference)
"""Trainium2 Bass kernel for nn_MultiDiscretePolicy.

Math:
  h   = relu(s @ W1 + b1)                         [B, 1024]
  aw  = h @ W2 + b2                               [B, 256]
  d   = aw @ Wd + db    (Wd = head_W[...,0]-head_W[...,1] transposed)
  out pair h: even = 1.0 if (logit0+g0) >= (logit1+g1) else 0.0, odd = 1-even
The reference's y + stop_grad(y_hard - y) is exactly one-hot in fp32, and
argmax(softmax(z)) == argmax(z), so the output reduces to the sign test
  even = (d >= gdn),  gdn = q0 - q1,  q_k = log(-log(u_k + EPS) + EPS)

Sharding: pure data parallel over the batch dim across 8 cores.  Matmuls keep
features on PSUM partitions / batch on the moving free dim, so the only
transpose needed (s^T) is done on host.  s and the weights are shipped as f16
(10-bit mantissa, on par with the PE's internal 12-bit f32r products) which
halves their HBM traffic; u stays f32 (the double-log gumbel transform
amplifies u rounding into comparison flips, so u cannot be narrowed).

Engine split per block: PE mm1/mm2/mm3; ACT gumbel Lns (p then q), most
relus(+b1) and half the awT bias-adds (Identity-with-bias); DVE the j6/j7
relus, the gdn subtracts, the other awT halves and ALL compares (d_ps is in
PSUM, which GPSIMD cannot read; DVE also has no divide, so the gumbel stays
in ln-subtract form).  PE is pre-warmed with tiny matmuls so the p-state
ramp completes while the first chunks stream in.  The DMA dispatch pipeline
costs ~1.2us/DMA (SP SEQ held through HWDGE), so chunk granularity is
size-dependent: merged 2-k chunks + late W2 at nb==1 (dispatch-bound, the
u->Ln chain is the tail), per-k chunks + early W2 at nb>1.
"""
from contextlib import ExitStack

import numpy as np

import concourse.bass as bass
import concourse.mybir as mybir
import concourse.tile as tile
from concourse import bacc
from concourse import bass_utils
from concourse.bass import ts, ds

N_CORES = 8
B, S_DIM, H_DIM, A_DIM = 32768, 1024, 1024, 512
D_HEAD = A_DIM // 2
EPS = 1e-20
BB = 512           # batch columns per block (one PSUM bank of fp32)
f32 = mybir.dt.float32
f16 = mybir.dt.float16
bf16 = mybir.dt.bfloat16
u8 = mybir.dt.uint8
AFT = mybir.ActivationFunctionType
OP = mybir.AluOpType

LAST_EXEC_NS = None

_cache: dict = {}


def _build_small(rpc: int, has_db: bool):
    """Single-block (rpc == BB) build, tuned for the dispatch-latency regime.

    Differences from the generic path:
      - u ships as uint16 fixed-point (u16 = round(u*65536)): absolute
        quantization (2^-17) beats f16's relative error exactly where the
        gumbel transform is sensitive (u near 1), at half the f32 bytes.
        The p-Ln folds the 2^-16 rescale into its scale argument.
      - DMA stream reordered so u quarters land mid-stream: the ACT Ln
        chain then finishes before the relu wave, letting ACT take half
        the relus (alternating DVE/ACT) so mm2 partials never stall.
      - gdn subtracts run on the otherwise-idle GPSIMD engine, taking
        them off DVE's tail-critical queue.
      - mm2 is split into two batch halves so awT/mm3/compare of half A
        pipeline against mm2 of half B on PE.
      - a tiny Ln right at the top hoists the 1.3us LoadActFuncSet off
        the Ln chain's critical path.
    """
    assert rpc == BB
    nc = bacc.Bacc("TRN2", target_bir_lowering=False, debug=False,
                   num_devices=N_CORES)

    # W1 and sT arrive host-interleaved per k-chunk as one tensor
    # M[s, :] = [W1[s, 0:512] | sT[s, :] | W1[s, 512:1024]] so each k chunk
    # is a single DMA, and k0 splits as (j0-3 weights + all sT) + (j4-7)
    # for the earliest possible PE start.
    Md = nc.dram_tensor("Md", [S_DIM, 3 * BB], f16, kind="ExternalInput").ap()
    u2d = nc.dram_tensor("u2", [rpc, 2 * A_DIM], mybir.dt.uint16,
                         kind="ExternalInput").ap()
    W2d = nc.dram_tensor("W2d", [128, 8 * D_HEAD], f16, kind="ExternalInput").ap()
    Wdd = nc.dram_tensor("Wdd", [128, 2 * A_DIM], f16, kind="ExternalInput").ap()
    bbd = nc.dram_tensor("bbd", [128, 10], f32, kind="ExternalInput").ap()
    idd = nc.dram_tensor("idd", [128, 128], f16, kind="ExternalInput").ap()
    if has_db:
        dbd = nc.dram_tensor("dbd", [1, A_DIM], f16, kind="ExternalInput").ap()
    outd = nc.dram_tensor("out", [rpc, A_DIM], u8, kind="ExternalOutput").ap()

    Mv = Md.rearrange("(a p) c -> p a c", p=128)            # [128, 8, 1536]
    u2v = u2d.rearrange("(q p h) m -> p q (h m)", p=128, h=2)  # [128,2,2048]
    outv = outd.rearrange("(g p x) m -> p g x m", p=128, x=4)

    with tile.TileContext(nc) as tc, ExitStack() as ctx:
        wp = ctx.enter_context(tc.tile_pool(name="weights", bufs=1))
        p_pool = ctx.enter_context(tc.tile_pool(name="pp", bufs=2))
        hT_pool = ctx.enter_context(tc.tile_pool(name="hTp", bufs=8))
        awT_pool = ctx.enter_context(tc.tile_pool(name="awTp", bufs=4))
        hd_psp = ctx.enter_context(tc.tile_pool(name="hdps", bufs=6, space="PSUM"))
        a_psp = ctx.enter_context(tc.tile_pool(name="aps", bufs=2, space="PSUM"))

        # ---- PE warmup: keeps the tensor engine continuously busy from
        # ~0.9us so the p-state ramp is done when real data lands (its
        # memset leads the DVE queue so warmups start earliest) ----
        warm_sb = wp.tile([128, 128], bf16)
        nc.vector.memset(warm_sb[:].bitcast(f32), 0.0)
        warm_ps = hd_psp.tile([128, BB], f32, name="warm_ps", tag="ps")
        for _ in range(26):
            nc.tensor.matmul(warm_ps[:, 0:128], warm_sb[:], warm_sb[:],
                             start=True, stop=True)

        # ---- table-load hoist: a 1-element Ln makes the framework emit
        # LoadActFuncSet here (~1us in) instead of before the first real Ln ----
        eps_sb = wp.tile([128, 1], f32)
        nc.vector.memset(eps_sb[:], EPS)
        dummy_sb = wp.tile([128, 1], f32)
        nc.scalar.activation(dummy_sb[:], eps_sb[:], AFT.Ln,
                             bias=eps_sb[:], scale=1.0)

        # ---- DMA stream (single SP queue; order == transfer order).
        # M[k] chunks pace PE's k-outer mm1; u halves interleave early so
        # the ACT Ln chain overlaps mm1 instead of trailing it ----
        M_sb = wp.tile([128, 8, 3 * BB], f16)
        u_sb = wp.tile([128, 4, 2 * A_DIM], mybir.dt.uint16)
        bb_sb = wp.tile([128, 10], f32)
        W2_sb = wp.tile([128, 8 * D_HEAD], f16)
        Wd_sb = wp.tile([128, 2 * A_DIM], f16)

        nc.sync.dma_start(M_sb[:, 0, 0:2 * BB], Mv[:, 0, 0:2 * BB])
        nc.sync.dma_start(M_sb[:, 0, 2 * BB:], Mv[:, 0, 2 * BB:])
        nc.sync.dma_start(M_sb[:, 1, :], Mv[:, 1, :])
        nc.sync.dma_start(M_sb[:, 2:4, :], Mv[:, 2:4, :])
        nc.sync.dma_start(u_sb[:, 0, :], u2v[:, 0, 0:2 * A_DIM])
        nc.sync.dma_start(M_sb[:, 4:6, :], Mv[:, 4:6, :])
        nc.sync.dma_start(u_sb[:, 1, :], u2v[:, 0, 2 * A_DIM:])
        nc.sync.dma_start(u_sb[:, 2, :], u2v[:, 1, 0:2 * A_DIM])
        nc.sync.dma_start(bb_sb[:], bbd[:])
        nc.sync.dma_start(M_sb[:, 6:8, :], Mv[:, 6:8, :])
        nc.sync.dma_start(u_sb[:, 3, :], u2v[:, 1, 2 * A_DIM:])
        nc.sync.dma_start(W2_sb[:], W2d[:])
        nc.sync.dma_start(Wd_sb[:], Wdd[:])
        id_sb = wp.tile([128, 128], f16)
        nc.sync.dma_start(id_sb[:], idd[:])
        if has_db:
            db_sb = wp.tile([1, A_DIM], f16)
            nc.sync.dma_start(db_sb[:], dbd[:])
            ones_sb = wp.tile([1, 128], f16)
            nc.vector.memset(ones_sb[:].bitcast(f32), 1.0)

        def W1k(k, j):
            off = j * 128 if j < 4 else 2 * BB + (j - 4) * 128
            return M_sb[:, k, ds(off, 128)]

        def sTk(k):
            return M_sb[:, k, ds(BB, BB)]

        b1_sb = bb_sb[:, 0:8]
        b2_sb = bb_sb[:, 8:10]
        W2_v = W2_sb.rearrange("p (j d) -> p j d", j=8)
        Wd_v = Wd_sb.rearrange("p (a m) -> p a m", a=2)

        # ---- gumbel per quarter: p = ln(u16*2^-16 + eps) and the
        # reference's q = ln(-p + eps) (== ln(-p) exactly in f32), both on
        # ACT; the pair subtract gdn = q0 - q1 runs on GPSIMD, which is
        # otherwise idle — DVE's tail queue stays free for relus/compares ----
        # bs0/bs2 keep gdn in f32 for DVE's is_ge compare; bs1/bs3 get the
        # NEGATED gdn in f16 — it is accumulated into d_ps by an identity
        # matmul so ACT's Sign (saturating u8 cast) yields the one-hot bit
        r_t = wp.tile([128, 4, 2 * A_DIM], f32)
        lnr_t = wp.tile([128, 4, A_DIM], f32)
        nlnr_t = wp.tile([128, 4, A_DIM], f16)
        for bs in range(4):
            p_t = p_pool.tile([128, 2 * A_DIM], f32, name="p_t")
            nc.scalar.activation(p_t[:], u_sb[:, bs, :], AFT.Ln,
                                 bias=eps_sb[:], scale=1.0 / 65536.0)
            q_t = r_t[:, bs, :]
            nc.scalar.activation(q_t, p_t[:], AFT.Ln,
                                 bias=eps_sb[:], scale=-1.0)
            if bs % 2 == 0:
                nc.gpsimd.tensor_tensor(lnr_t[:, bs, :],
                                        q_t[0:128, 0::2], q_t[0:128, 1::2],
                                        OP.subtract)
            else:
                nc.gpsimd.tensor_tensor(nlnr_t[:, bs, :],
                                        q_t[0:128, 1::2], q_t[0:128, 0::2],
                                        OP.subtract)

        # ---- mm1: single 8-wide k-outer; j6/j7 borrow the two a-bank
        # slots (freed by their relus before mm2's first accumulate).
        # k0 runs j0-3 first (they only need the first half-chunk M0a) ----
        h_pss = [(hd_psp if j < 6 else a_psp).tile(
            [128, BB], f32, name="h_ps",
            tag="ps" if j < 6 else "a") for j in range(8)]
        for k in range(8):
            jorder = (6, 7, 0, 1, 2, 3, 4, 5) if k == 7 else range(8)
            for j in jorder:
                nc.tensor.matmul(h_pss[j][:], W1k(k, j),
                                 sTk(k), start=(k == 0), stop=(k == 7))

        # ---- relus: 4 on DVE (free once gdn moved to GPSIMD), 4 on ACT
        # after its Ln chain drains — both cadences beat mm2's 2-matmul-
        # per-j consumption ----
        hT_ts = [None] * 8
        for i, j in enumerate((6, 7, 0, 1, 2, 3, 4, 5)):
            hT_t = hT_pool.tile([128, BB], f16, name="hT_t")
            if i < 4:
                nc.vector.tensor_scalar(hT_t[:], h_pss[j][:],
                                        b1_sb[:, j:j + 1], 0.0,
                                        OP.add, OP.max)
            else:
                nc.scalar.activation(hT_t[:], h_pss[j][:], AFT.Relu,
                                     bias=b1_sb[:, j:j + 1], scale=1.0)
            hT_ts[j] = hT_t

        # ---- mm2 in relu-completion order, dt0's 16 partials first: its
        # awT then runs mid-mm2, so mm3's dt0 sweep starts the moment mm2
        # ends (readers of a PSUM bank must wait for the bank's full
        # accumulation group, so no batch split) ----
        a_pss = [a_psp.tile([128, BB], f32, name="a_ps", tag="a")
                 for _ in range(2)]
        HB = BB // 2
        for dt_ in range(2):
            for j in (6, 7, 0, 1, 2, 3, 4, 5):
                nc.tensor.matmul(
                    a_pss[dt_][:], W2_v[:, j, ts(dt_, 128)],
                    hT_ts[j][:], start=(j == 6), stop=(j == 5),
                    skip_group_check=True)

        # awT bias-adds split per half and across DVE/ACT
        awT_ts = [[None, None], [None, None]]
        for hf in range(2):
            for dt_ in range(2):
                awT_t = awT_pool.tile([128, HB], f16, name="awT_t")
                a_half = a_pss[dt_][:, ds(hf * HB, HB)]
                if dt_ == 0:
                    nc.vector.tensor_scalar_add(awT_t[:], a_half,
                                                b2_sb[:, dt_:dt_ + 1])
                else:
                    nc.scalar.activation(awT_t[:], a_half, AFT.Identity,
                                         bias=b2_sb[:, dt_:dt_ + 1],
                                         scale=1.0)
                awT_ts[dt_][hf] = awT_t

        # ---- mm3 + compare per 128-row group: all dt0 partials sweep
        # first (their awT is ready mid-mm2), then per-bs dt1(+identity)
        # stops feed the compares.  bs0/bs2 compare on DVE (is_ge vs lnr),
        # bs1/bs3 fold -gdn into PSUM via an identity matmul and emit on
        # ACT (Sign -> saturating u8 {0,1}), so the two engines drain the
        # compare chain in parallel ----
        o_t = wp.tile([128, 4, A_DIM], u8)
        d_pss = [hd_psp.tile([128, A_DIM], f32, name="d_ps", tag="ps")
                 for _ in range(4)]
        for bs in range(4):
            nc.tensor.matmul(
                d_pss[bs][:], awT_ts[0][bs // 2][:, ts(bs % 2, 128)],
                Wd_v[:, 0, :], start=True, stop=False)
        for bs in range(4):
            sgn = bs % 2 == 1
            d_ps = d_pss[bs]
            nc.tensor.matmul(
                d_ps[:], awT_ts[1][bs // 2][:, ts(bs % 2, 128)],
                Wd_v[:, 1, :], start=False,
                stop=not has_db and not sgn)
            if has_db:
                nc.tensor.matmul(d_ps[:], ones_sb[:], db_sb[:],
                                 start=False, stop=not sgn)
            if sgn:
                nc.tensor.matmul(d_ps[:], id_sb[:], nlnr_t[:, bs, :],
                                 start=False, stop=True)
                nc.scalar.activation(o_t[:, bs, :], d_ps[:], AFT.Sign,
                                     bias=eps_sb[:], scale=1.0)
            else:
                nc.vector.tensor_tensor(o_t[:, bs, :], d_ps[:],
                                        lnr_t[:, bs, :], OP.is_ge)
        # split the tail store so only 64KB trails the last compare
        nc.sync.dma_start(outv[:, 0, 0:2, :], o_t[:, 0:2, :])
        nc.sync.dma_start(outv[:, 0, 2:3, :], o_t[:, 2:3, :])
        nc.sync.dma_start(outv[:, 0, 3:4, :], o_t[:, 3:4, :])

    nc.compile()
    return nc


def _build(rpc: int, has_db: bool):
    """Build the per-core kernel for `rpc` batch rows per core."""
    nb = rpc // BB
    nc = bacc.Bacc("TRN2", target_bir_lowering=False, debug=False,
                   num_devices=N_CORES)

    sTd = nc.dram_tensor("sT", [S_DIM, rpc], f16, kind="ExternalInput").ap()
    u2d = nc.dram_tensor("u2", [rpc, 2 * A_DIM], mybir.dt.uint16,
                         kind="ExternalInput").ap()
    W1d = nc.dram_tensor("W1d", [S_DIM, H_DIM], f16, kind="ExternalInput").ap()
    # W2/Wd arrive host-packed partition-major so one partition's data is a
    # single contiguous run (DMA chunk size drives HBM efficiency)
    W2d = nc.dram_tensor("W2d", [128, 8 * D_HEAD], f16, kind="ExternalInput").ap()
    Wdd = nc.dram_tensor("Wdd", [128, 2 * A_DIM], f16, kind="ExternalInput").ap()
    bbd = nc.dram_tensor("bbd", [128, 10], f32, kind="ExternalInput").ap()
    if has_db:
        dbd = nc.dram_tensor("dbd", [1, A_DIM], f16, kind="ExternalInput").ap()
    # only the even elements of each output pair are shipped (odd = 1 - even),
    # as uint8 {0,1} — exact, since the fp32 output is exactly one-hot
    outd = nc.dram_tensor("out", [rpc, A_DIM], u8, kind="ExternalOutput").ap()

    sTv = sTd.rearrange("(a p) b -> p a b", p=128)      # [128, 8, rpc]
    # u arrives host-permuted in row pairs: row = q*256 + p*2 + h, so each
    # partition line of a pair-load is 8KB contiguous
    u2v = u2d.rearrange("(q p h) m -> p q (h m)", p=128, h=2)
    # out leaves partition-major within each 2-block group: DRAM row
    # g*(128*x) + p*x + xx, so each group store writes x*512 contiguous
    # bytes per partition (host undoes the permutation)
    xgrp = 8 if nb % 2 == 0 else 4
    assert nb % 2 == 0 or nb == 1
    outv = outd.rearrange("(g p x) m -> p g x m", p=128, x=xgrp)

    with tile.TileContext(nc) as tc, ExitStack() as ctx:
        wp = ctx.enter_context(tc.tile_pool(name="weights", bufs=1))
        sT_pool = ctx.enter_context(tc.tile_pool(name="sTp", bufs=4))
        u_pool = ctx.enter_context(tc.tile_pool(name="up", bufs=3))
        p_pool = ctx.enter_context(tc.tile_pool(name="pp", bufs=1))
        lnr_pool = ctx.enter_context(tc.tile_pool(name="lnrp", bufs=3))
        gdn_pool = ctx.enter_context(tc.tile_pool(name="gdnp", bufs=2))
        hT_pool = ctx.enter_context(tc.tile_pool(name="hTp", bufs=10))
        awT_pool = ctx.enter_context(tc.tile_pool(name="awTp", bufs=6))
        out_pool = ctx.enter_context(tc.tile_pool(name="outp", bufs=3))
        # h and d tiles share one 6-slot tag so mm1 (block0's k-outer) and
        # mm3 time-share PSUM banks; awT holds the other 2 banks.
        hd_psp = ctx.enter_context(tc.tile_pool(name="hdps", bufs=6, space="PSUM"))
        a_psp = ctx.enter_context(tc.tile_pool(name="aps", bufs=2, space="PSUM"))

        W1v = W1d.rearrange("(a p) j -> p a j", p=128)

        ngrp = (nb + 1) // 2

        def g_cols(g):
            return min(2 * BB, rpc - g * 2 * BB)

        # ---- PE warmup: tiny bf16 matmuls so the p-state ramp runs while
        # the first real chunks are still in flight ----
        warm_sb = wp.tile([128, 128], bf16)
        nc.vector.memset(warm_sb[:].bitcast(f32), 0.0)
        warm_ps = hd_psp.tile([128, BB], f32, name="warm_ps", tag="ps")
        for _ in range(22):
            nc.tensor.matmul(warm_ps[:, 0:128], warm_sb[:], warm_sb[:],
                             start=True, stop=True)

        # ---- DMA stream (SP queue order == transfer order) ----
        bb_sb = wp.tile([128, 10], f32)
        nc.sync.dma_start(bb_sb[:], bbd[:])
        b1_sb = bb_sb[:, 0:8]
        b2_sb = bb_sb[:, 8:10]

        W1_sb = wp.tile([128, 8, H_DIM], f16)
        sT0_ts = []
        for ka in range(2):
            sT_t = sT_pool.tile([128, 4, 2 * BB], f16, name="sT_t")
            sT0_ts.append(sT_t)
        u_ts_all = [None] * nb

        def load_u(b0, uq=None):
            if uq is None or uq == 0:
                u_t = u_pool.tile([128, 4, 2 * A_DIM], mybir.dt.uint16,
                                  name="u_t")
                u_ts_all[b0] = u_t
            else:
                u_t = u_ts_all[b0]
            if uq is None:
                nc.sync.dma_start(
                    u_t[:].rearrange("p (q h) m -> p q (h m)", q=2),
                    u2v[:, 2 * b0:2 * b0 + 2, :])
            else:
                nc.sync.dma_start(
                    u_t[:, uq, :],
                    u2v[:, 2 * b0 + uq // 2,
                        ds((uq % 2) * 2 * A_DIM, 2 * A_DIM)])

        # block0 chunks: W1/sT stream fine-grained at the head (earliest PE
        # start), then 2-k-wide merged DMAs — the DMA dispatch pipeline
        # costs ~1.2us per DMA regardless of size, so small chunks waste
        # dispatch slots and push the u quarters (and the ACT Ln chain) late
        nc.sync.dma_start(W1_sb[:, 0, 0:256], W1v[:, 0, 0:256])
        nc.sync.dma_start(sT0_ts[0][:, 0, 0:g_cols(0)],
                          sTv[:, 0, ds(0, g_cols(0))])
        nc.sync.dma_start(W1_sb[:, 0, 256:1024], W1v[:, 0, 256:1024])
        nc.sync.dma_start(W1_sb[:, 1, :], W1v[:, 1, :])
        nc.sync.dma_start(sT0_ts[0][:, 1, 0:g_cols(0)],
                          sTv[:, 1, ds(0, g_cols(0))])
        if nb == 1:
            # merged 2-k chunks: at this size the stream is dispatch-bound
            # (~1.2us per DMA), not bandwidth-bound; the first sT pair stays
            # split so k2's column isn't gated on k3's bytes
            for k0_ in (2, 4, 6):
                nc.sync.dma_start(W1_sb[:, k0_:k0_ + 2, :],
                                  W1v[:, k0_:k0_ + 2, :])
                for kc in (k0_, k0_ + 1):
                    nc.sync.dma_start(sT0_ts[kc // 4][:, kc % 4, 0:g_cols(0)],
                                      sTv[:, kc, ds(0, g_cols(0))])
        else:
            # per-k chunks: 1MB sT slices here, so finer grain keeps mm1 fed
            for k in range(2, 8):
                nc.sync.dma_start(W1_sb[:, k, :], W1v[:, k, :])
                nc.sync.dma_start(sT0_ts[k // 4][:, k % 4, 0:g_cols(0)],
                                  sTv[:, k, ds(0, g_cols(0))])
        # nb==1: W2 late (mm2 is late, u's Ln chain is the critical tail);
        # nb>1: W2 before u (block0's mm2 comes sooner)
        W2_sb = wp.tile([128, 8 * D_HEAD], f16)
        W2_v = W2_sb.rearrange("p (j d) -> p j d", j=8)
        if nb > 1:
            nc.sync.dma_start(W2_sb[:], W2d[:])
        for uq in range(3):
            load_u(0, uq=uq)
        if nb == 1:
            nc.sync.dma_start(W2_sb[:], W2d[:])
        load_u(0, uq=3)
        Wd_sb = wp.tile([128, 2 * A_DIM], f16)
        nc.sync.dma_start(Wd_sb[:], Wdd[:])
        Wd_v = Wd_sb.rearrange("p (a m) -> p a m", a=2)
        eps_sb = wp.tile([128, 1], f32)
        nc.vector.memset(eps_sb[:], EPS)
        if has_db:
            db_sb = wp.tile([1, A_DIM], f16)
            nc.sync.dma_start(db_sb[:], dbd[:])
            ones_sb = wp.tile([1, 128], f16)
            nc.vector.memset(ones_sb[:].bitcast(f32), 1.0)
        if nb > 1:
            load_u(1)

        cur_sT = sT0_ts
        next_sT = None
        o_t = None

        def sT_group_load(g):
            tiles = []
            for ka in range(2):
                sT_t = sT_pool.tile([128, 4, 2 * BB], f16, name="sT_t")
                nc.sync.dma_start(
                    sT_t[:, :, 0:g_cols(g)],
                    sTv[:, ka * 4:(ka + 1) * 4, ds(g * 2 * BB, g_cols(g))])
                tiles.append(sT_t)
            return tiles

        for b0 in range(nb):
            g = b0 // 2
            bw = b0 % 2
            if bw == 0:
                o_t = out_pool.tile([128, 8, A_DIM], u8, name="o_t")
            else:
                # prefetch the NEXT 2-block sT group one block ahead — late
                # enough to keep the head DMA queue short, early enough that
                # the 2MB lands within one block period
                if g + 1 < ngrp:
                    next_sT = sT_group_load(g + 1)
            if b0 + 2 < nb:
                load_u(b0 + 2)

            u_t = u_ts_all[b0]

            def sT_at(k):
                return cur_sT[k // 4][:, k % 4, ds(bw * BB, BB)]

            # ---- gumbel, ratio-domain.  p = ln(u+eps) (< 0 always), and the
            # reference's q = ln(-p+eps) satisfies -p+eps == -p exactly in
            # f32 (|p| >= 6e-8 >> 1e-20), so
            #   gdn = q0-q1 = ln(E0/E1) = ln(p0/p1) = ln(R).
            # One Ln pass + a strided divide (Pool) + a half-size Ln replace
            # Ln+Ln+subtract; ACT only ever runs Ln/Relu/Identity, which
            # share one activation table (no LoadActFuncSet churn). ----
            r_t = gdn_pool.tile([128, 4, 2 * A_DIM], f32, name="r_t")
            lnr_t = lnr_pool.tile([128, 4, A_DIM], f32, name="lnr_t")

            def q_quarter(bs):
                # block0: per-bs gumbel so each quarter's chain starts the
                # moment its u DMA lands
                p_t = p_pool.tile([128, 2 * A_DIM], f32, name="p_t")
                nc.scalar.activation(p_t[:], u_t[:, bs, :], AFT.Ln,
                                     bias=eps_sb[:], scale=1.0 / 65536.0)
                q_t = r_t[:, bs, :]
                nc.scalar.activation(q_t, p_t[:], AFT.Ln,
                                     bias=eps_sb[:], scale=-1.0)

            def gdn_quarter(bs):
                q_t = r_t[:, bs, :]
                nc.vector.tensor_tensor(lnr_t[:, bs, :],
                                        q_t[0:128, 0::2], q_t[0:128, 1::2],
                                        OP.subtract)

            if b0 == 0:
                for bs in range(4):
                    q_quarter(bs)
                gdn_quarter(0)
                gdn_quarter(1)
            else:
                p_t = p_pool.tile([128, 4, 2 * A_DIM], f32, name="p_t")
                nc.scalar.activation(
                    p_t[:].rearrange("p h m -> p (h m)"),
                    u_t[:].rearrange("p h m -> p (h m)"),
                    AFT.Ln, bias=eps_sb[:], scale=1.0 / 65536.0)
                nc.scalar.activation(
                    r_t[:].rearrange("p q m -> p (q m)"),
                    p_t[:].rearrange("p h m -> p (h m)"),
                    AFT.Ln, bias=eps_sb[:], scale=-1.0)

            # ---- mm1 (+ mm2 interleaved one j-group behind for b0>0) ----
            hT_ts = [None] * 8

            def relu(j, h_ps, eng):
                hT_t = hT_pool.tile([128, BB], f16, name="hT_t")
                if eng == "act":
                    nc.scalar.activation(hT_t[:], h_ps[:], AFT.Relu,
                                         bias=b1_sb[:, j:j + 1], scale=1.0)
                else:
                    nc.vector.tensor_scalar(hT_t[:], h_ps[:],
                                            b1_sb[:, j:j + 1], 0.0,
                                            OP.add, OP.max)
                hT_ts[j] = hT_t

            def mm2_partial(j, j_start=0, j_stop=7):
                for dt_ in range(2):
                    nc.tensor.matmul(a_pss[dt_][:], W2_v[:, j, ts(dt_, 128)],
                                     hT_ts[j][:], start=(j == j_start),
                                     stop=(j == j_stop), skip_group_check=True)

            if b0 == 0:
                # single 8-wide k-outer: j6/j7 borrow the two a-bank slots
                # (freed by their relus before mm2's first accumulate), so PE
                # streams mm1 with no group barrier.  k7's column runs j6/j7
                # first so the relu pipeline frees those banks earliest.
                h_pss = [(hd_psp if j < 6 else a_psp).tile(
                    [128, BB], f32, name="h_ps",
                    tag="ps" if j < 6 else "a") for j in range(8)]
                for k in range(8):
                    jorder = (6, 7, 0, 1, 2, 3, 4, 5) if k == 7 else range(8)
                    for j in jorder:
                        nc.tensor.matmul(
                            h_pss[j][:], W1_sb[:, k, ts(j, 128)],
                            sT_at(k), start=(k == 0), stop=(k == 7))
                # relus: DVE leads (ACT drains the u Lns first, then takes
                # the tail); emitted j6/j7-first to free the borrowed a-banks
                for i, j in enumerate((6, 7, 0, 1, 2, 3, 4, 5)):
                    relu(j, h_pss[j], "dve" if i < 6 else "act")
                a_pss = [a_psp.tile([128, BB], f32, name="a_ps", tag="a")
                         for _ in range(2)]
                # accumulate mm2 in relu-completion order so its first
                # matmuls fire as soon as relu j6 lands
                for j in (6, 7, 0, 1, 2, 3, 4, 5):
                    mm2_partial(j, j_start=6, j_stop=5)
            else:
                a_pss = [a_psp.tile([128, BB], f32, name="a_ps", tag="a")
                         for _ in range(2)]
                # last block: run j6/j7 first so their DVE relus finish
                # during mm1 and the tail's mm2 partials wait only on ACT
                # relus that are already done by mm1's end
                last = b0 == nb - 1
                jorder = (6, 7, 0, 1, 2, 3, 4, 5) if last else range(8)
                js, je = (6, 5) if last else (0, 7)
                for i, j in enumerate(jorder):
                    # the first j borrows an a-bank slot (free across the
                    # block boundary) so this block's first mm1 writes never
                    # wait on the previous block's d_ps compares
                    h_ps = (a_psp if i == 0 else hd_psp).tile(
                        [128, BB], f32, name="h_ps",
                        tag="a" if i == 0 else "ps")
                    for k in range(8):
                        nc.tensor.matmul(h_ps[:], W1_sb[:, k, ts(j, 128)],
                                         sT_at(k), start=(k == 0),
                                         stop=(k == 7))
                    # j6/j7 relus on DVE so the whole-block lnR Ln (queued
                    # on ACT after relu j5) never delays mm2's tail
                    relu(j, h_ps, "act" if j < 6 else "dve")
                    if i == 5:
                        nc.vector.tensor_tensor(
                            lnr_t[:], r_t[:, :, 0::2], r_t[:, :, 1::2],
                            OP.subtract)
                    if i >= 2:
                        mm2_partial(jorder[i - 2] if last else j - 2,
                                    j_start=js, j_stop=je)
                mm2_partial(jorder[6] if last else 6, j_start=js, j_stop=je)
                mm2_partial(jorder[7] if last else 7, j_start=js, j_stop=je)

            # awT split per half and across DVE/ACT so mm3's first matmul
            # waits only ~0.4us of bias-add, not the full-width pair
            awT_ts = [[None, None], [None, None]]
            for hf in range(2):
                for dt_ in range(2):
                    awT_t = awT_pool.tile([128, BB // 2], f16, name="awT_t")
                    a_half = a_pss[dt_][:, ds(hf * (BB // 2), BB // 2)]
                    if dt_ == 0:
                        nc.vector.tensor_scalar_add(awT_t[:], a_half,
                                                    b2_sb[:, dt_:dt_ + 1])
                    else:
                        nc.scalar.activation(awT_t[:], a_half, AFT.Identity,
                                             bias=b2_sb[:, dt_:dt_ + 1],
                                             scale=1.0)
                    awT_ts[dt_][hf] = awT_t
            if b0 == 0:
                gdn_quarter(2)
                gdn_quarter(3)

            # ---- mm3 + compare/emit per 128-row group (d_ps lives in
            # PSUM, so the compares must run on DVE) ----
            for bs in range(4):
                d_ps = hd_psp.tile([128, A_DIM], f32, name="d_ps", tag="ps")
                for dt_ in range(2):
                    nc.tensor.matmul(
                        d_ps[:], awT_ts[dt_][bs // 2][:, ts(bs % 2, 128)],
                        Wd_v[:, dt_, :], start=(dt_ == 0),
                        stop=(dt_ == 1 and not has_db))
                if has_db:
                    nc.tensor.matmul(d_ps[:], ones_sb[:], db_sb[:],
                                     start=False, stop=True)
                nc.vector.tensor_tensor(o_t[:, bw * 4 + bs, :], d_ps[:],
                                        lnr_t[:, bs, :], OP.is_ge)
            last_of_group = (bw == 1) or (b0 == nb - 1)
            if last_of_group:
                nx = 4 * (bw + 1)
                if b0 == nb - 1:
                    # split the tail store so only 64KB trails the last
                    # compare
                    nc.sync.dma_start(outv[:, g, 0:nx - 2, :],
                                      o_t[:, 0:nx - 2, :])
                    nc.sync.dma_start(outv[:, g, nx - 2:nx - 1, :],
                                      o_t[:, nx - 2:nx - 1, :])
                    nc.sync.dma_start(outv[:, g, nx - 1:nx, :],
                                      o_t[:, nx - 1:nx, :])
                else:
                    nc.sync.dma_start(outv[:, g, 0:nx, :], o_t[:, 0:nx, :])
                cur_sT = next_sT

    nc.compile()
    return nc


def kernel(s, u, W1, b1, W2, b2, head_W, head_b, _rpc=None):
    global LAST_EXEC_NS
    s = np.asarray(s, dtype=np.float32)
    u = np.asarray(u, dtype=np.float32)
    W1 = np.asarray(W1, dtype=np.float32)
    W2 = np.asarray(W2, dtype=np.float32)
    b1 = np.asarray(b1, dtype=np.float32)
    b2 = np.asarray(b2, dtype=np.float32)
    head_W = np.asarray(head_W, dtype=np.float32)
    head_b = np.asarray(head_b, dtype=np.float32)

    nrows = s.shape[0]
    rpc = _rpc if _rpc is not None else nrows // N_CORES
    assert nrows == rpc * N_CORES and rpc % BB == 0

    sT = np.ascontiguousarray(s.T.astype(np.float16))   # [S_DIM, nrows] f16
    # u as uint16 fixed-point: round(u * 2^16), clamped.  Absolute error
    # 2^-17 beats f16's relative error where the gumbel transform is
    # sensitive (u near 1) at the same 2 bytes/elem.
    u2 = np.minimum(np.round(u.reshape(nrows, 2 * A_DIM) * 65536.0),
                    65535.0).astype(np.uint16)
    Wd = np.ascontiguousarray((head_W[:, :, 0] - head_W[:, :, 1]).T)
    W1h = W1.astype(np.float16)
    W2h = np.ascontiguousarray(
        W2.astype(np.float16).reshape(8, 128, D_HEAD).transpose(1, 0, 2)
    ).reshape(128, 8 * D_HEAD)
    Wdh = np.ascontiguousarray(
        Wd.astype(np.float16).reshape(2, 128, A_DIM).transpose(1, 0, 2)
    ).reshape(128, 2 * A_DIM)
    db = np.ascontiguousarray(head_b[:, 0] - head_b[:, 1]).reshape(1, A_DIM)
    has_db = bool(np.any(db))
    bb = np.concatenate([b1.reshape(8, 128).T, b2.reshape(2, 128).T], axis=1)
    bb = np.ascontiguousarray(bb, dtype=np.float32)     # [128, 10]

    key = (rpc, has_db)
    if key not in _cache:
        _cache[key] = (_build_small(rpc, has_db) if rpc == BB
                       else _build(rpc, has_db))
    nc = _cache[key]

    nq = rpc // 256
    in_maps = []
    for c in range(N_CORES):
        uc = u2[c * rpc:(c + 1) * rpc]
        up = np.ascontiguousarray(
            uc.reshape(nq, 2, 128, 2 * A_DIM).transpose(0, 2, 1, 3)
        ).reshape(rpc, 2 * A_DIM)
        m = {
            "u2": up,
            "W2d": W2h, "Wdd": Wdh, "bbd": bb,
        }
        if rpc == BB:
            # small path: W1 and sT interleaved per k chunk as
            # [W1 j0-3 | sT | W1 j4-7]
            m["Md"] = np.ascontiguousarray(np.concatenate(
                [W1h[:, 0:512], sT[:, c * rpc:(c + 1) * rpc],
                 W1h[:, 512:1024]], axis=1))
            m["idd"] = np.eye(128, dtype=np.float16)
        else:
            m["sT"] = np.ascontiguousarray(sT[:, c * rpc:(c + 1) * rpc])
            m["W1d"] = W1h
        if has_db:
            m["dbd"] = db.astype(np.float16)
        in_maps.append(m)

    res = bass_utils.run_bass_kernel_spmd(nc, in_maps,
                                          core_ids=list(range(N_CORES)))
    LAST_EXEC_NS = res.exec_time_ns
    nb = rpc // BB
    xgrp = 8 if nb % 2 == 0 else 4
    shards = []
    for c in range(N_CORES):
        e = res.results[c]["out"]                        # [rpc, A_DIM] uint8
        # undo the (g, p, x) store permutation back to batch order
        e = e.reshape(rpc // (128 * xgrp), 128, xgrp, A_DIM)
        shards.append(e.transpose(0, 2, 1, 3).reshape(rpc, A_DIM))
    evens = np.concatenate(shards, axis=0)               # [nrows, A_DIM]
    out = np.empty((nrows, 2 * A_DIM), dtype=np.float32)
    ef = evens.astype(np.float32)
    out[:, 0::2] = ef
    out[:, 1::2] = 1.0 - ef
    return out



# revision 25
# speedup vs baseline: 1.0082x; 1.0082x over previous
"""Trainium2 Bass kernel for nn_MultiDiscretePolicy.

Math:
  h   = relu(s @ W1 + b1)                         [B, 1024]
  aw  = h @ W2 + b2                               [B, 256]
  d   = aw @ Wd + db    (Wd = head_W[...,0]-head_W[...,1] transposed)
  out pair h: even = 1.0 if (logit0+g0) >= (logit1+g1) else 0.0, odd = 1-even
The reference's y + stop_grad(y_hard - y) is exactly one-hot in fp32, and
argmax(softmax(z)) == argmax(z), so the output reduces to the sign test
  even = (d >= gdn),  gdn = q0 - q1,  q_k = log(-log(u_k + EPS) + EPS)

Sharding: pure data parallel over the batch dim across 8 cores.  Matmuls keep
features on PSUM partitions / batch on the moving free dim, so the only
transpose needed (s^T) is done on host.  s and the weights are shipped as f16
(10-bit mantissa, on par with the PE's internal 12-bit f32r products) which
halves their HBM traffic; u stays f32 (the double-log gumbel transform
amplifies u rounding into comparison flips, so u cannot be narrowed).

Engine split per block: PE mm1/mm2/mm3; ACT gumbel Lns (p then q), most
relus(+b1) and half the awT bias-adds (Identity-with-bias); DVE the j6/j7
relus, the gdn subtracts, the other awT halves and ALL compares (d_ps is in
PSUM, which GPSIMD cannot read; DVE also has no divide, so the gumbel stays
in ln-subtract form).  PE is pre-warmed with tiny matmuls so the p-state
ramp completes while the first chunks stream in.  The DMA dispatch pipeline
costs ~1.2us/DMA (SP SEQ held through HWDGE), so chunk granularity is
size-dependent: merged 2-k chunks + late W2 at nb==1 (dispatch-bound, the
u->Ln chain is the tail), per-k chunks + early W2 at nb>1.
"""
from contextlib import ExitStack

import numpy as np

import concourse.bass as bass
import concourse.mybir as mybir
import concourse.tile as tile
from concourse import bacc
from concourse import bass_utils
from concourse.bass import ts, ds

N_CORES = 8
B, S_DIM, H_DIM, A_DIM = 32768, 1024, 1024, 512
D_HEAD = A_DIM // 2
EPS = 1e-20
BB = 512           # batch columns per block (one PSUM bank of fp32)
f32 = mybir.dt.float32
f16 = mybir.dt.float16
bf16 = mybir.dt.bfloat16
u8 = mybir.dt.uint8
AFT = mybir.ActivationFunctionType
OP = mybir.AluOpType

LAST_EXEC_NS = None

_cache: dict = {}


def _build_small(rpc: int, has_db: bool):
    """Single-block (rpc == BB) build, tuned for the dispatch-latency regime.

    Differences from the generic path:
      - u ships as uint16 fixed-point (u16 = round(u*65536)): absolute
        quantization (2^-17) beats f16's relative error exactly where the
        gumbel transform is sensitive (u near 1), at half the f32 bytes.
        The p-Ln folds the 2^-16 rescale into its scale argument.
      - DMA stream reordered so u quarters land mid-stream: the ACT Ln
        chain then finishes before the relu wave, letting ACT take half
        the relus (alternating DVE/ACT) so mm2 partials never stall.
      - gdn subtracts run on the otherwise-idle GPSIMD engine, taking
        them off DVE's tail-critical queue.
      - mm2 is split into two batch halves so awT/mm3/compare of half A
        pipeline against mm2 of half B on PE.
      - a tiny Ln right at the top hoists the 1.3us LoadActFuncSet off
        the Ln chain's critical path.
    """
    assert rpc == BB
    nc = bacc.Bacc("TRN2", target_bir_lowering=False, debug=False,
                   num_devices=N_CORES)

    # W1 and sT arrive host-interleaved per k-chunk as one tensor
    # M[s, :] = [W1[s, 0:512] | sT[s, :] | W1[s, 512:1024]] so each k chunk
    # is a single DMA, and k0 splits as (j0-3 weights + all sT) + (j4-7)
    # for the earliest possible PE start.
    Md = nc.dram_tensor("Md", [S_DIM, 3 * BB], f16, kind="ExternalInput").ap()
    u2d = nc.dram_tensor("u2", [rpc, 2 * A_DIM], mybir.dt.uint16,
                         kind="ExternalInput").ap()
    W2d = nc.dram_tensor("W2d", [128, 8 * D_HEAD], f16, kind="ExternalInput").ap()
    Wdd = nc.dram_tensor("Wdd", [128, 2 * A_DIM], f16, kind="ExternalInput").ap()
    bbd = nc.dram_tensor("bbd", [128, 10], f32, kind="ExternalInput").ap()
    idd = nc.dram_tensor("idd", [128, 128], f16, kind="ExternalInput").ap()
    if has_db:
        dbd = nc.dram_tensor("dbd", [1, A_DIM], f16, kind="ExternalInput").ap()
    outd = nc.dram_tensor("out", [rpc, A_DIM], u8, kind="ExternalOutput").ap()

    Mv = Md.rearrange("(a p) c -> p a c", p=128)            # [128, 8, 1536]
    u2v = u2d.rearrange("(q p h) m -> p q (h m)", p=128, h=2)  # [128,2,2048]
    outv = outd.rearrange("(g p x) m -> p g x m", p=128, x=4)

    with tile.TileContext(nc) as tc, ExitStack() as ctx:
        wp = ctx.enter_context(tc.tile_pool(name="weights", bufs=1))
        p_pool = ctx.enter_context(tc.tile_pool(name="pp", bufs=2))
        hT_pool = ctx.enter_context(tc.tile_pool(name="hTp", bufs=8))
        awT_pool = ctx.enter_context(tc.tile_pool(name="awTp", bufs=4))
        hd_psp = ctx.enter_context(tc.tile_pool(name="hdps", bufs=6, space="PSUM"))
        a_psp = ctx.enter_context(tc.tile_pool(name="aps", bufs=2, space="PSUM"))

        # ---- PE warmup: keeps the tensor engine continuously busy from
        # ~0.9us so the p-state ramp is done when real data lands (its
        # memset leads the DVE queue so warmups start earliest) ----
        warm_sb = wp.tile([128, 128], bf16)
        nc.vector.memset(warm_sb[:].bitcast(f32), 0.0)
        warm_ps = hd_psp.tile([128, BB], f32, name="warm_ps", tag="ps")
        for _ in range(26):
            nc.tensor.matmul(warm_ps[:, 0:128], warm_sb[:], warm_sb[:],
                             start=True, stop=True)

        # ---- table-load hoist: a 1-element Ln makes the framework emit
        # LoadActFuncSet here (~1us in) instead of before the first real Ln ----
        eps_sb = wp.tile([128, 1], f32)
        nc.vector.memset(eps_sb[:], EPS)
        dummy_sb = wp.tile([128, 1], f32)
        nc.scalar.activation(dummy_sb[:], eps_sb[:], AFT.Ln,
                             bias=eps_sb[:], scale=1.0)

        # ---- DMA stream (single SP queue; order == transfer order).
        # M[k] chunks pace PE's k-outer mm1; u halves interleave early so
        # the ACT Ln chain overlaps mm1 instead of trailing it ----
        M_sb = wp.tile([128, 8, 3 * BB], f16)
        u_sb = wp.tile([128, 4, 2 * A_DIM], mybir.dt.uint16)
        bb_sb = wp.tile([128, 10], f32)
        W2_sb = wp.tile([128, 8 * D_HEAD], f16)
        Wd_sb = wp.tile([128, 2 * A_DIM], f16)

        nc.sync.dma_start(M_sb[:, 0, 0:2 * BB], Mv[:, 0, 0:2 * BB])
        nc.sync.dma_start(M_sb[:, 0, 2 * BB:], Mv[:, 0, 2 * BB:])
        nc.sync.dma_start(M_sb[:, 1, :], Mv[:, 1, :])
        nc.sync.dma_start(M_sb[:, 2:4, :], Mv[:, 2:4, :])
        nc.sync.dma_start(u_sb[:, 0, :], u2v[:, 0, 0:2 * A_DIM])
        nc.sync.dma_start(M_sb[:, 4:6, :], Mv[:, 4:6, :])
        nc.sync.dma_start(u_sb[:, 1, :], u2v[:, 0, 2 * A_DIM:])
        nc.sync.dma_start(u_sb[:, 2, :], u2v[:, 1, 0:2 * A_DIM])
        nc.sync.dma_start(bb_sb[:], bbd[:])
        nc.sync.dma_start(M_sb[:, 6:8, :], Mv[:, 6:8, :])
        nc.sync.dma_start(u_sb[:, 3, :], u2v[:, 1, 2 * A_DIM:])
        nc.sync.dma_start(W2_sb[:], W2d[:])
        nc.sync.dma_start(Wd_sb[:], Wdd[:])
        id_sb = wp.tile([128, 128], f16)
        nc.sync.dma_start(id_sb[:], idd[:])
        if has_db:
            db_sb = wp.tile([1, A_DIM], f16)
            nc.sync.dma_start(db_sb[:], dbd[:])
            ones_sb = wp.tile([1, 128], f16)
            nc.vector.memset(ones_sb[:].bitcast(f32), 1.0)

        def W1k(k, j):
            off = j * 128 if j < 4 else 2 * BB + (j - 4) * 128
            return M_sb[:, k, ds(off, 128)]

        def sTk(k):
            return M_sb[:, k, ds(BB, BB)]

        b1_sb = bb_sb[:, 0:8]
        b2_sb = bb_sb[:, 8:10]
        W2_v = W2_sb.rearrange("p (j d) -> p j d", j=8)
        Wd_v = Wd_sb.rearrange("p (a m) -> p a m", a=2)

        # ---- gumbel per quarter: p = ln(u16*2^-16 + eps) and the
        # reference's q = ln(-p + eps) (== ln(-p) exactly in f32), both on
        # ACT; the pair subtract gdn = q0 - q1 runs on GPSIMD, which is
        # otherwise idle — DVE's tail queue stays free for relus/compares ----
        # bs0/bs2 keep gdn in f32 for DVE's is_ge compare; bs1/bs3 get the
        # NEGATED gdn in f16 — it is accumulated into d_ps by an identity
        # matmul so ACT's Sign (saturating u8 cast) yields the one-hot bit
        r_t = wp.tile([128, 4, 2 * A_DIM], f32)
        lnr_t = wp.tile([128, 4, A_DIM], f32)
        nlnr_t = wp.tile([128, 4, A_DIM], f16)
        for bs in range(4):
            p_t = p_pool.tile([128, 2 * A_DIM], f32, name="p_t")
            nc.scalar.activation(p_t[:], u_sb[:, bs, :], AFT.Ln,
                                 bias=eps_sb[:], scale=1.0 / 65536.0)
            q_t = r_t[:, bs, :]
            nc.scalar.activation(q_t, p_t[:], AFT.Ln,
                                 bias=eps_sb[:], scale=-1.0)
            if bs % 2 == 0:
                nc.gpsimd.tensor_tensor(lnr_t[:, bs, :],
                                        q_t[0:128, 0::2], q_t[0:128, 1::2],
                                        OP.subtract)
            else:
                nc.gpsimd.tensor_tensor(nlnr_t[:, bs, :],
                                        q_t[0:128, 1::2], q_t[0:128, 0::2],
                                        OP.subtract)

        # ---- mm1: single 8-wide k-outer; j6/j7 borrow the two a-bank
        # slots (freed by their relus before mm2's first accumulate).
        # k0 runs j0-3 first (they only need the first half-chunk M0a) ----
        h_pss = [(hd_psp if j < 6 else a_psp).tile(
            [128, BB], f32, name="h_ps",
            tag="ps" if j < 6 else "a") for j in range(8)]
        for k in range(8):
            jorder = (6, 7, 0, 1, 2, 3, 4, 5) if k == 7 else range(8)
            for j in jorder:
                nc.tensor.matmul(h_pss[j][:], W1k(k, j),
                                 sTk(k), start=(k == 0), stop=(k == 7))

        # ---- relus, split DVE/ACT so each lands before mm2 consumes it:
        # DVE starts at mm1's k7 column (j6 first), ACT joins once its Ln
        # chain drains ----
        hT_ts = [None] * 8
        for j in (6, 7, 0, 1, 2, 3, 4, 5):
            hT_t = hT_pool.tile([128, BB], f16, name="hT_t")
            if j in (6, 7, 1, 4):
                nc.vector.tensor_scalar(hT_t[:], h_pss[j][:],
                                        b1_sb[:, j:j + 1], 0.0,
                                        OP.add, OP.max)
            else:
                nc.scalar.activation(hT_t[:], h_pss[j][:], AFT.Relu,
                                     bias=b1_sb[:, j:j + 1], scale=1.0)
            hT_ts[j] = hT_t

        # ---- mm2 in relu-completion order, dt-interleaved except the
        # tail: j4/j5's dt0 partials run before their dt1 twins so
        # a_ps[0] stops two matmuls early — its awT then overlaps mm2's
        # tail and mm3's dt0 sweep starts the moment mm2 ends ----
        a_pss = [a_psp.tile([128, BB], f32, name="a_ps", tag="a")
                 for _ in range(2)]
        HB = BB // 2
        for j in (6, 7, 0, 1, 2, 3):
            for dt_ in range(2):
                nc.tensor.matmul(
                    a_pss[dt_][:], W2_v[:, j, ts(dt_, 128)],
                    hT_ts[j][:], start=(j == 6), stop=False,
                    skip_group_check=True)
        for dt_ in range(2):
            for j in (4, 5):
                nc.tensor.matmul(
                    a_pss[dt_][:], W2_v[:, j, ts(dt_, 128)],
                    hT_ts[j][:], start=False, stop=(j == 5),
                    skip_group_check=True)

        # awT bias-adds split per half and across DVE/ACT
        awT_ts = [[None, None], [None, None]]
        for hf in range(2):
            for dt_ in range(2):
                awT_t = awT_pool.tile([128, HB], f16, name="awT_t")
                a_half = a_pss[dt_][:, ds(hf * HB, HB)]
                if dt_ == 0:
                    nc.vector.tensor_scalar_add(awT_t[:], a_half,
                                                b2_sb[:, dt_:dt_ + 1])
                else:
                    nc.scalar.activation(awT_t[:], a_half, AFT.Identity,
                                         bias=b2_sb[:, dt_:dt_ + 1],
                                         scale=1.0)
                awT_ts[dt_][hf] = awT_t

        # ---- mm3 + compare per 128-row group: all dt0 partials sweep
        # first (their awT is ready mid-mm2), then per-bs dt1(+identity)
        # stops feed the compares.  bs0/bs2 compare on DVE (is_ge vs lnr),
        # bs1/bs3 fold -gdn into PSUM via an identity matmul and emit on
        # ACT (Sign -> saturating u8 {0,1}), so the two engines drain the
        # compare chain in parallel ----
        o_t = wp.tile([128, 4, A_DIM], u8)
        d_pss = [hd_psp.tile([128, A_DIM], f32, name="d_ps", tag="ps")
                 for _ in range(4)]
        for bs in range(4):
            nc.tensor.matmul(
                d_pss[bs][:], awT_ts[0][bs // 2][:, ts(bs % 2, 128)],
                Wd_v[:, 0, :], start=True, stop=False)
        for bs in range(4):
            sgn = bs % 2 == 1
            d_ps = d_pss[bs]
            nc.tensor.matmul(
                d_ps[:], awT_ts[1][bs // 2][:, ts(bs % 2, 128)],
                Wd_v[:, 1, :], start=False,
                stop=not has_db and not sgn)
            if has_db:
                nc.tensor.matmul(d_ps[:], ones_sb[:], db_sb[:],
                                 start=False, stop=not sgn)
            if sgn:
                nc.tensor.matmul(d_ps[:], id_sb[:], nlnr_t[:, bs, :],
                                 start=False, stop=True)
                nc.scalar.activation(o_t[:, bs, :], d_ps[:], AFT.Sign,
                                     bias=eps_sb[:], scale=1.0)
            else:
                nc.vector.tensor_tensor(o_t[:, bs, :], d_ps[:],
                                        lnr_t[:, bs, :], OP.is_ge)
        # split the tail store so only 64KB trails the last compare
        nc.sync.dma_start(outv[:, 0, 0:2, :], o_t[:, 0:2, :])
        nc.sync.dma_start(outv[:, 0, 2:3, :], o_t[:, 2:3, :])
        nc.sync.dma_start(outv[:, 0, 3:4, :], o_t[:, 3:4, :])

    nc.compile()
    return nc


def _build(rpc: int, has_db: bool):
    """Build the per-core kernel for `rpc` batch rows per core."""
    nb = rpc // BB
    nc = bacc.Bacc("TRN2", target_bir_lowering=False, debug=False,
                   num_devices=N_CORES)

    sTd = nc.dram_tensor("sT", [S_DIM, rpc], f16, kind="ExternalInput").ap()
    u2d = nc.dram_tensor("u2", [rpc, 2 * A_DIM], mybir.dt.uint16,
                         kind="ExternalInput").ap()
    W1d = nc.dram_tensor("W1d", [S_DIM, H_DIM], f16, kind="ExternalInput").ap()
    # W2/Wd arrive host-packed partition-major so one partition's data is a
    # single contiguous run (DMA chunk size drives HBM efficiency)
    W2d = nc.dram_tensor("W2d", [128, 8 * D_HEAD], f16, kind="ExternalInput").ap()
    Wdd = nc.dram_tensor("Wdd", [128, 2 * A_DIM], f16, kind="ExternalInput").ap()
    bbd = nc.dram_tensor("bbd", [128, 10], f32, kind="ExternalInput").ap()
    if has_db:
        dbd = nc.dram_tensor("dbd", [1, A_DIM], f16, kind="ExternalInput").ap()
    # only the even elements of each output pair are shipped (odd = 1 - even),
    # as uint8 {0,1} — exact, since the fp32 output is exactly one-hot
    outd = nc.dram_tensor("out", [rpc, A_DIM], u8, kind="ExternalOutput").ap()

    sTv = sTd.rearrange("(a p) b -> p a b", p=128)      # [128, 8, rpc]
    # u arrives host-permuted in row pairs: row = q*256 + p*2 + h, so each
    # partition line of a pair-load is 8KB contiguous
    u2v = u2d.rearrange("(q p h) m -> p q (h m)", p=128, h=2)
    # out leaves partition-major within each 2-block group: DRAM row
    # g*(128*x) + p*x + xx, so each group store writes x*512 contiguous
    # bytes per partition (host undoes the permutation)
    xgrp = 8 if nb % 2 == 0 else 4
    assert nb % 2 == 0 or nb == 1
    outv = outd.rearrange("(g p x) m -> p g x m", p=128, x=xgrp)

    with tile.TileContext(nc) as tc, ExitStack() as ctx:
        wp = ctx.enter_context(tc.tile_pool(name="weights", bufs=1))
        sT_pool = ctx.enter_context(tc.tile_pool(name="sTp", bufs=4))
        u_pool = ctx.enter_context(tc.tile_pool(name="up", bufs=3))
        p_pool = ctx.enter_context(tc.tile_pool(name="pp", bufs=1))
        lnr_pool = ctx.enter_context(tc.tile_pool(name="lnrp", bufs=3))
        gdn_pool = ctx.enter_context(tc.tile_pool(name="gdnp", bufs=2))
        hT_pool = ctx.enter_context(tc.tile_pool(name="hTp", bufs=10))
        awT_pool = ctx.enter_context(tc.tile_pool(name="awTp", bufs=6))
        out_pool = ctx.enter_context(tc.tile_pool(name="outp", bufs=3))
        # h and d tiles share one 6-slot tag so mm1 (block0's k-outer) and
        # mm3 time-share PSUM banks; awT holds the other 2 banks.
        hd_psp = ctx.enter_context(tc.tile_pool(name="hdps", bufs=6, space="PSUM"))
        a_psp = ctx.enter_context(tc.tile_pool(name="aps", bufs=2, space="PSUM"))

        W1v = W1d.rearrange("(a p) j -> p a j", p=128)

        ngrp = (nb + 1) // 2

        def g_cols(g):
            return min(2 * BB, rpc - g * 2 * BB)

        # ---- PE warmup: tiny bf16 matmuls so the p-state ramp runs while
        # the first real chunks are still in flight ----
        warm_sb = wp.tile([128, 128], bf16)
        nc.vector.memset(warm_sb[:].bitcast(f32), 0.0)
        warm_ps = hd_psp.tile([128, BB], f32, name="warm_ps", tag="ps")
        for _ in range(22):
            nc.tensor.matmul(warm_ps[:, 0:128], warm_sb[:], warm_sb[:],
                             start=True, stop=True)

        # ---- DMA stream (SP queue order == transfer order) ----
        bb_sb = wp.tile([128, 10], f32)
        nc.sync.dma_start(bb_sb[:], bbd[:])
        b1_sb = bb_sb[:, 0:8]
        b2_sb = bb_sb[:, 8:10]

        W1_sb = wp.tile([128, 8, H_DIM], f16)
        sT0_ts = []
        for ka in range(2):
            sT_t = sT_pool.tile([128, 4, 2 * BB], f16, name="sT_t")
            sT0_ts.append(sT_t)
        u_ts_all = [None] * nb

        def load_u(b0, uq=None):
            if uq is None or uq == 0:
                u_t = u_pool.tile([128, 4, 2 * A_DIM], mybir.dt.uint16,
                                  name="u_t")
                u_ts_all[b0] = u_t
            else:
                u_t = u_ts_all[b0]
            if uq is None:
                nc.sync.dma_start(
                    u_t[:].rearrange("p (q h) m -> p q (h m)", q=2),
                    u2v[:, 2 * b0:2 * b0 + 2, :])
            else:
                nc.sync.dma_start(
                    u_t[:, uq, :],
                    u2v[:, 2 * b0 + uq // 2,
                        ds((uq % 2) * 2 * A_DIM, 2 * A_DIM)])

        # block0 chunks: W1/sT stream fine-grained at the head (earliest PE
        # start), then 2-k-wide merged DMAs — the DMA dispatch pipeline
        # costs ~1.2us per DMA regardless of size, so small chunks waste
        # dispatch slots and push the u quarters (and the ACT Ln chain) late
        nc.sync.dma_start(W1_sb[:, 0, 0:256], W1v[:, 0, 0:256])
        nc.sync.dma_start(sT0_ts[0][:, 0, 0:g_cols(0)],
                          sTv[:, 0, ds(0, g_cols(0))])
        nc.sync.dma_start(W1_sb[:, 0, 256:1024], W1v[:, 0, 256:1024])
        nc.sync.dma_start(W1_sb[:, 1, :], W1v[:, 1, :])
        nc.sync.dma_start(sT0_ts[0][:, 1, 0:g_cols(0)],
                          sTv[:, 1, ds(0, g_cols(0))])
        if nb == 1:
            # merged 2-k chunks: at this size the stream is dispatch-bound
            # (~1.2us per DMA), not bandwidth-bound; the first sT pair stays
            # split so k2's column isn't gated on k3's bytes
            for k0_ in (2, 4, 6):
                nc.sync.dma_start(W1_sb[:, k0_:k0_ + 2, :],
                                  W1v[:, k0_:k0_ + 2, :])
                for kc in (k0_, k0_ + 1):
                    nc.sync.dma_start(sT0_ts[kc // 4][:, kc % 4, 0:g_cols(0)],
                                      sTv[:, kc, ds(0, g_cols(0))])
        else:
            # per-k chunks: 1MB sT slices here, so finer grain keeps mm1 fed
            for k in range(2, 8):
                nc.sync.dma_start(W1_sb[:, k, :], W1v[:, k, :])
                nc.sync.dma_start(sT0_ts[k // 4][:, k % 4, 0:g_cols(0)],
                                  sTv[:, k, ds(0, g_cols(0))])
        # nb==1: W2 late (mm2 is late, u's Ln chain is the critical tail);
        # nb>1: W2 before u (block0's mm2 comes sooner)
        W2_sb = wp.tile([128, 8 * D_HEAD], f16)
        W2_v = W2_sb.rearrange("p (j d) -> p j d", j=8)
        if nb > 1:
            nc.sync.dma_start(W2_sb[:], W2d[:])
        for uq in range(3):
            load_u(0, uq=uq)
        if nb == 1:
            nc.sync.dma_start(W2_sb[:], W2d[:])
        load_u(0, uq=3)
        Wd_sb = wp.tile([128, 2 * A_DIM], f16)
        nc.sync.dma_start(Wd_sb[:], Wdd[:])
        Wd_v = Wd_sb.rearrange("p (a m) -> p a m", a=2)
        eps_sb = wp.tile([128, 1], f32)
        nc.vector.memset(eps_sb[:], EPS)
        if has_db:
            db_sb = wp.tile([1, A_DIM], f16)
            nc.sync.dma_start(db_sb[:], dbd[:])
            ones_sb = wp.tile([1, 128], f16)
            nc.vector.memset(ones_sb[:].bitcast(f32), 1.0)
        if nb > 1:
            load_u(1)

        cur_sT = sT0_ts
        next_sT = None
        o_t = None

        def sT_group_load(g):
            tiles = []
            for ka in range(2):
                sT_t = sT_pool.tile([128, 4, 2 * BB], f16, name="sT_t")
                nc.sync.dma_start(
                    sT_t[:, :, 0:g_cols(g)],
                    sTv[:, ka * 4:(ka + 1) * 4, ds(g * 2 * BB, g_cols(g))])
                tiles.append(sT_t)
            return tiles

        for b0 in range(nb):
            g = b0 // 2
            bw = b0 % 2
            if bw == 0:
                o_t = out_pool.tile([128, 8, A_DIM], u8, name="o_t")
            else:
                # prefetch the NEXT 2-block sT group one block ahead — late
                # enough to keep the head DMA queue short, early enough that
                # the 2MB lands within one block period
                if g + 1 < ngrp:
                    next_sT = sT_group_load(g + 1)
            if b0 + 2 < nb:
                load_u(b0 + 2)

            u_t = u_ts_all[b0]

            def sT_at(k):
                return cur_sT[k // 4][:, k % 4, ds(bw * BB, BB)]

            # ---- gumbel, ratio-domain.  p = ln(u+eps) (< 0 always), and the
            # reference's q = ln(-p+eps) satisfies -p+eps == -p exactly in
            # f32 (|p| >= 6e-8 >> 1e-20), so
            #   gdn = q0-q1 = ln(E0/E1) = ln(p0/p1) = ln(R).
            # One Ln pass + a strided divide (Pool) + a half-size Ln replace
            # Ln+Ln+subtract; ACT only ever runs Ln/Relu/Identity, which
            # share one activation table (no LoadActFuncSet churn). ----
            r_t = gdn_pool.tile([128, 4, 2 * A_DIM], f32, name="r_t")
            lnr_t = lnr_pool.tile([128, 4, A_DIM], f32, name="lnr_t")

            def q_quarter(bs):
                # block0: per-bs gumbel so each quarter's chain starts the
                # moment its u DMA lands
                p_t = p_pool.tile([128, 2 * A_DIM], f32, name="p_t")
                nc.scalar.activation(p_t[:], u_t[:, bs, :], AFT.Ln,
                                     bias=eps_sb[:], scale=1.0 / 65536.0)
                q_t = r_t[:, bs, :]
                nc.scalar.activation(q_t, p_t[:], AFT.Ln,
                                     bias=eps_sb[:], scale=-1.0)

            def gdn_quarter(bs):
                q_t = r_t[:, bs, :]
                nc.vector.tensor_tensor(lnr_t[:, bs, :],
                                        q_t[0:128, 0::2], q_t[0:128, 1::2],
                                        OP.subtract)

            if b0 == 0:
                for bs in range(4):
                    q_quarter(bs)
                gdn_quarter(0)
                gdn_quarter(1)
            else:
                p_t = p_pool.tile([128, 4, 2 * A_DIM], f32, name="p_t")
                nc.scalar.activation(
                    p_t[:].rearrange("p h m -> p (h m)"),
                    u_t[:].rearrange("p h m -> p (h m)"),
                    AFT.Ln, bias=eps_sb[:], scale=1.0 / 65536.0)
                nc.scalar.activation(
                    r_t[:].rearrange("p q m -> p (q m)"),
                    p_t[:].rearrange("p h m -> p (h m)"),
                    AFT.Ln, bias=eps_sb[:], scale=-1.0)

            # ---- mm1 (+ mm2 interleaved one j-group behind for b0>0) ----
            hT_ts = [None] * 8

            def relu(j, h_ps, eng):
                hT_t = hT_pool.tile([128, BB], f16, name="hT_t")
                if eng == "act":
                    nc.scalar.activation(hT_t[:], h_ps[:], AFT.Relu,
                                         bias=b1_sb[:, j:j + 1], scale=1.0)
                else:
                    nc.vector.tensor_scalar(hT_t[:], h_ps[:],
                                            b1_sb[:, j:j + 1], 0.0,
                                            OP.add, OP.max)
                hT_ts[j] = hT_t

            def mm2_partial(j, j_start=0, j_stop=7):
                for dt_ in range(2):
                    nc.tensor.matmul(a_pss[dt_][:], W2_v[:, j, ts(dt_, 128)],
                                     hT_ts[j][:], start=(j == j_start),
                                     stop=(j == j_stop), skip_group_check=True)

            if b0 == 0:
                # single 8-wide k-outer: j6/j7 borrow the two a-bank slots
                # (freed by their relus before mm2's first accumulate), so PE
                # streams mm1 with no group barrier.  k7's column runs j6/j7
                # first so the relu pipeline frees those banks earliest.
                h_pss = [(hd_psp if j < 6 else a_psp).tile(
                    [128, BB], f32, name="h_ps",
                    tag="ps" if j < 6 else "a") for j in range(8)]
                for k in range(8):
                    jorder = (6, 7, 0, 1, 2, 3, 4, 5) if k == 7 else range(8)
                    for j in jorder:
                        nc.tensor.matmul(
                            h_pss[j][:], W1_sb[:, k, ts(j, 128)],
                            sT_at(k), start=(k == 0), stop=(k == 7))
                # relus: DVE leads (ACT drains the u Lns first, then takes
                # the tail); emitted j6/j7-first to free the borrowed a-banks
                for i, j in enumerate((6, 7, 0, 1, 2, 3, 4, 5)):
                    relu(j, h_pss[j], "dve" if i < 6 else "act")
                a_pss = [a_psp.tile([128, BB], f32, name="a_ps", tag="a")
                         for _ in range(2)]
                # accumulate mm2 in relu-completion order so its first
                # matmuls fire as soon as relu j6 lands
                for j in (6, 7, 0, 1, 2, 3, 4, 5):
                    mm2_partial(j, j_start=6, j_stop=5)
            else:
                a_pss = [a_psp.tile([128, BB], f32, name="a_ps", tag="a")
                         for _ in range(2)]
                # last block: run j6/j7 first so their DVE relus finish
                # during mm1 and the tail's mm2 partials wait only on ACT
                # relus that are already done by mm1's end
                last = b0 == nb - 1
                jorder = (6, 7, 0, 1, 2, 3, 4, 5) if last else range(8)
                js, je = (6, 5) if last else (0, 7)
                for i, j in enumerate(jorder):
                    # the first j borrows an a-bank slot (free across the
                    # block boundary) so this block's first mm1 writes never
                    # wait on the previous block's d_ps compares
                    h_ps = (a_psp if i == 0 else hd_psp).tile(
                        [128, BB], f32, name="h_ps",
                        tag="a" if i == 0 else "ps")
                    for k in range(8):
                        nc.tensor.matmul(h_ps[:], W1_sb[:, k, ts(j, 128)],
                                         sT_at(k), start=(k == 0),
                                         stop=(k == 7))
                    # j6/j7 relus on DVE so the whole-block lnR Ln (queued
                    # on ACT after relu j5) never delays mm2's tail
                    relu(j, h_ps, "act" if j < 6 else "dve")
                    if i == 5:
                        nc.vector.tensor_tensor(
                            lnr_t[:], r_t[:, :, 0::2], r_t[:, :, 1::2],
                            OP.subtract)
                    if i >= 2:
                        mm2_partial(jorder[i - 2] if last else j - 2,
                                    j_start=js, j_stop=je)
                mm2_partial(jorder[6] if last else 6, j_start=js, j_stop=je)
                mm2_partial(jorder[7] if last else 7, j_start=js, j_stop=je)

            # awT split per half and across DVE/ACT so mm3's first matmul
            # waits only ~0.4us of bias-add, not the full-width pair
            awT_ts = [[None, None], [None, None]]
            for hf in range(2):
                for dt_ in range(2):
                    awT_t = awT_pool.tile([128, BB // 2], f16, name="awT_t")
                    a_half = a_pss[dt_][:, ds(hf * (BB // 2), BB // 2)]
                    if dt_ == 0:
                        nc.vector.tensor_scalar_add(awT_t[:], a_half,
                                                    b2_sb[:, dt_:dt_ + 1])
                    else:
                        nc.scalar.activation(awT_t[:], a_half, AFT.Identity,
                                             bias=b2_sb[:, dt_:dt_ + 1],
                                             scale=1.0)
                    awT_ts[dt_][hf] = awT_t
            if b0 == 0:
                gdn_quarter(2)
                gdn_quarter(3)

            # ---- mm3 + compare/emit per 128-row group (d_ps lives in
            # PSUM, so the compares must run on DVE) ----
            for bs in range(4):
                d_ps = hd_psp.tile([128, A_DIM], f32, name="d_ps", tag="ps")
                for dt_ in range(2):
                    nc.tensor.matmul(
                        d_ps[:], awT_ts[dt_][bs // 2][:, ts(bs % 2, 128)],
                        Wd_v[:, dt_, :], start=(dt_ == 0),
                        stop=(dt_ == 1 and not has_db))
                if has_db:
                    nc.tensor.matmul(d_ps[:], ones_sb[:], db_sb[:],
                                     start=False, stop=True)
                nc.vector.tensor_tensor(o_t[:, bw * 4 + bs, :], d_ps[:],
                                        lnr_t[:, bs, :], OP.is_ge)
            last_of_group = (bw == 1) or (b0 == nb - 1)
            if last_of_group:
                nx = 4 * (bw + 1)
                if b0 == nb - 1:
                    # split the tail store so only 64KB trails the last
                    # compare
                    nc.sync.dma_start(outv[:, g, 0:nx - 2, :],
                                      o_t[:, 0:nx - 2, :])
                    nc.sync.dma_start(outv[:, g, nx - 2:nx - 1, :],
                                      o_t[:, nx - 2:nx - 1, :])
                    nc.sync.dma_start(outv[:, g, nx - 1:nx, :],
                                      o_t[:, nx - 1:nx, :])
                else:
                    nc.sync.dma_start(outv[:, g, 0:nx, :], o_t[:, 0:nx, :])
                cur_sT = next_sT

    nc.compile()
    return nc


def kernel(s, u, W1, b1, W2, b2, head_W, head_b, _rpc=None):
    global LAST_EXEC_NS
    s = np.asarray(s, dtype=np.float32)
    u = np.asarray(u, dtype=np.float32)
    W1 = np.asarray(W1, dtype=np.float32)
    W2 = np.asarray(W2, dtype=np.float32)
    b1 = np.asarray(b1, dtype=np.float32)
    b2 = np.asarray(b2, dtype=np.float32)
    head_W = np.asarray(head_W, dtype=np.float32)
    head_b = np.asarray(head_b, dtype=np.float32)

    nrows = s.shape[0]
    rpc = _rpc if _rpc is not None else nrows // N_CORES
    assert nrows == rpc * N_CORES and rpc % BB == 0

    sT = np.ascontiguousarray(s.T.astype(np.float16))   # [S_DIM, nrows] f16
    # u as uint16 fixed-point: round(u * 2^16), clamped.  Absolute error
    # 2^-17 beats f16's relative error where the gumbel transform is
    # sensitive (u near 1) at the same 2 bytes/elem.
    u2 = np.minimum(np.round(u.reshape(nrows, 2 * A_DIM) * 65536.0),
                    65535.0).astype(np.uint16)
    Wd = np.ascontiguousarray((head_W[:, :, 0] - head_W[:, :, 1]).T)
    W1h = W1.astype(np.float16)
    W2h = np.ascontiguousarray(
        W2.astype(np.float16).reshape(8, 128, D_HEAD).transpose(1, 0, 2)
    ).reshape(128, 8 * D_HEAD)
    Wdh = np.ascontiguousarray(
        Wd.astype(np.float16).reshape(2, 128, A_DIM).transpose(1, 0, 2)
    ).reshape(128, 2 * A_DIM)
    db = np.ascontiguousarray(head_b[:, 0] - head_b[:, 1]).reshape(1, A_DIM)
    has_db = bool(np.any(db))
    bb = np.concatenate([b1.reshape(8, 128).T, b2.reshape(2, 128).T], axis=1)
    bb = np.ascontiguousarray(bb, dtype=np.float32)     # [128, 10]

    key = (rpc, has_db)
    if key not in _cache:
        _cache[key] = (_build_small(rpc, has_db) if rpc == BB
                       else _build(rpc, has_db))
    nc = _cache[key]

    nq = rpc // 256
    in_maps = []
    for c in range(N_CORES):
        uc = u2[c * rpc:(c + 1) * rpc]
        up = np.ascontiguousarray(
            uc.reshape(nq, 2, 128, 2 * A_DIM).transpose(0, 2, 1, 3)
        ).reshape(rpc, 2 * A_DIM)
        m = {
            "u2": up,
            "W2d": W2h, "Wdd": Wdh, "bbd": bb,
        }
        if rpc == BB:
            # small path: W1 and sT interleaved per k chunk as
            # [W1 j0-3 | sT | W1 j4-7]
            m["Md"] = np.ascontiguousarray(np.concatenate(
                [W1h[:, 0:512], sT[:, c * rpc:(c + 1) * rpc],
                 W1h[:, 512:1024]], axis=1))
            m["idd"] = np.eye(128, dtype=np.float16)
        else:
            m["sT"] = np.ascontiguousarray(sT[:, c * rpc:(c + 1) * rpc])
            m["W1d"] = W1h
        if has_db:
            m["dbd"] = db.astype(np.float16)
        in_maps.append(m)

    res = bass_utils.run_bass_kernel_spmd(nc, in_maps,
                                          core_ids=list(range(N_CORES)))
    LAST_EXEC_NS = res.exec_time_ns
    nb = rpc // BB
    xgrp = 8 if nb % 2 == 0 else 4
    shards = []
    for c in range(N_CORES):
        e = res.results[c]["out"]                        # [rpc, A_DIM] uint8
        # undo the (g, p, x) store permutation back to batch order
        e = e.reshape(rpc // (128 * xgrp), 128, xgrp, A_DIM)
        shards.append(e.transpose(0, 2, 1, 3).reshape(rpc, A_DIM))
    evens = np.concatenate(shards, axis=0)               # [nrows, A_DIM]
    out = np.empty((nrows, 2 * A_DIM), dtype=np.float32)
    ef = evens.astype(np.float32)
    out[:, 0::2] = ef
    out[:, 1::2] = 1.0 - ef
    return out



# revision 27
# speedup vs baseline: 1.0194x; 1.0111x over previous
"""Trainium2 Bass kernel for nn_MultiDiscretePolicy.

Math:
  h   = relu(s @ W1 + b1)                         [B, 1024]
  aw  = h @ W2 + b2                               [B, 256]
  d   = aw @ Wd + db    (Wd = head_W[...,0]-head_W[...,1] transposed)
  out pair h: even = 1.0 if (logit0+g0) >= (logit1+g1) else 0.0, odd = 1-even
The reference's y + stop_grad(y_hard - y) is exactly one-hot in fp32, and
argmax(softmax(z)) == argmax(z), so the output reduces to the sign test
  even = (d >= gdn),  gdn = q0 - q1,  q_k = log(-log(u_k + EPS) + EPS)

Sharding: pure data parallel over the batch dim across 8 cores.  Matmuls keep
features on PSUM partitions / batch on the moving free dim, so the only
transpose needed (s^T) is done on host.  s and the weights are shipped as f16
(10-bit mantissa, on par with the PE's internal 12-bit f32r products) which
halves their HBM traffic; u stays f32 (the double-log gumbel transform
amplifies u rounding into comparison flips, so u cannot be narrowed).

Engine split per block: PE mm1/mm2/mm3; ACT gumbel Lns (p then q), most
relus(+b1) and half the awT bias-adds (Identity-with-bias); DVE the j6/j7
relus, the gdn subtracts, the other awT halves and ALL compares (d_ps is in
PSUM, which GPSIMD cannot read; DVE also has no divide, so the gumbel stays
in ln-subtract form).  PE is pre-warmed with tiny matmuls so the p-state
ramp completes while the first chunks stream in.  The DMA dispatch pipeline
costs ~1.2us/DMA (SP SEQ held through HWDGE), so chunk granularity is
size-dependent: merged 2-k chunks + late W2 at nb==1 (dispatch-bound, the
u->Ln chain is the tail), per-k chunks + early W2 at nb>1.
"""
from contextlib import ExitStack

import numpy as np

import concourse.bass as bass
import concourse.mybir as mybir
import concourse.tile as tile
from concourse import bacc
from concourse import bass_utils
from concourse.bass import ts, ds

N_CORES = 8
B, S_DIM, H_DIM, A_DIM = 32768, 1024, 1024, 512
D_HEAD = A_DIM // 2
EPS = 1e-20
BB = 512           # batch columns per block (one PSUM bank of fp32)
f32 = mybir.dt.float32
f16 = mybir.dt.float16
bf16 = mybir.dt.bfloat16
u8 = mybir.dt.uint8
AFT = mybir.ActivationFunctionType
OP = mybir.AluOpType

LAST_EXEC_NS = None

_cache: dict = {}


def _build_small(rpc: int, has_db: bool):
    """Single-block (rpc == BB) build, tuned for the dispatch-latency regime.

    Differences from the generic path:
      - u ships as uint16 fixed-point (u16 = round(u*65536)): absolute
        quantization (2^-17) beats f16's relative error exactly where the
        gumbel transform is sensitive (u near 1), at half the f32 bytes.
        The p-Ln folds the 2^-16 rescale into its scale argument.
      - DMA stream reordered so u quarters land mid-stream: the ACT Ln
        chain then finishes before the relu wave, letting ACT take half
        the relus (alternating DVE/ACT) so mm2 partials never stall.
      - gdn subtracts run on the otherwise-idle GPSIMD engine, taking
        them off DVE's tail-critical queue.
      - mm2 is split into two batch halves so awT/mm3/compare of half A
        pipeline against mm2 of half B on PE.
      - a tiny Ln right at the top hoists the 1.3us LoadActFuncSet off
        the Ln chain's critical path.
    """
    assert rpc == BB
    nc = bacc.Bacc("TRN2", target_bir_lowering=False, debug=False,
                   num_devices=N_CORES)

    # W1 and sT arrive host-interleaved per k-chunk as one tensor
    # M[s, :] = [W1[s, 0:512] | sT[s, :] | W1[s, 512:1024]] so each k chunk
    # is a single DMA, and k0 splits as (j0-3 weights + all sT) + (j4-7)
    # for the earliest possible PE start.
    Md = nc.dram_tensor("Md", [S_DIM, 3 * BB], f16, kind="ExternalInput").ap()
    u2d = nc.dram_tensor("u2", [rpc, 2 * A_DIM], mybir.dt.uint16,
                         kind="ExternalInput").ap()
    W2d = nc.dram_tensor("W2d", [128, 8 * D_HEAD], f16, kind="ExternalInput").ap()
    Wdd = nc.dram_tensor("Wdd", [128, 2 * A_DIM], f16, kind="ExternalInput").ap()
    bbd = nc.dram_tensor("bbd", [128, 10], f32, kind="ExternalInput").ap()
    idd = nc.dram_tensor("idd", [128, 128], f16, kind="ExternalInput").ap()
    if has_db:
        dbd = nc.dram_tensor("dbd", [1, A_DIM], f16, kind="ExternalInput").ap()
    outd = nc.dram_tensor("out", [rpc, A_DIM], u8, kind="ExternalOutput").ap()

    Mv = Md.rearrange("(a p) c -> p a c", p=128)            # [128, 8, 1536]
    u2v = u2d.rearrange("(q p h) m -> p q (h m)", p=128, h=2)  # [128,2,2048]
    outv = outd.rearrange("(g p x) m -> p g x m", p=128, x=4)

    with tile.TileContext(nc) as tc, ExitStack() as ctx:
        wp = ctx.enter_context(tc.tile_pool(name="weights", bufs=1))
        p_pool = ctx.enter_context(tc.tile_pool(name="pp", bufs=2))
        hT_pool = ctx.enter_context(tc.tile_pool(name="hTp", bufs=8))
        awT_pool = ctx.enter_context(tc.tile_pool(name="awTp", bufs=4))
        hd_psp = ctx.enter_context(tc.tile_pool(name="hdps", bufs=6, space="PSUM"))
        a_psp = ctx.enter_context(tc.tile_pool(name="aps", bufs=2, space="PSUM"))

        # ---- PE warmup: keeps the tensor engine continuously busy from
        # ~0.9us so the p-state ramp is done when real data lands (its
        # memset leads the DVE queue so warmups start earliest) ----
        warm_sb = wp.tile([128, 128], bf16)
        nc.vector.memset(warm_sb[:].bitcast(f32), 0.0)
        warm_ps = hd_psp.tile([128, BB], f32, name="warm_ps", tag="ps")
        for _ in range(26):
            nc.tensor.matmul(warm_ps[:, 0:128], warm_sb[:], warm_sb[:],
                             start=True, stop=True)

        # ---- table-load hoist: a 1-element Ln makes the framework emit
        # LoadActFuncSet here (~1us in) instead of before the first real Ln ----
        eps_sb = wp.tile([128, 1], f32)
        nc.vector.memset(eps_sb[:], EPS)
        dummy_sb = wp.tile([128, 1], f32)
        nc.scalar.activation(dummy_sb[:], eps_sb[:], AFT.Ln,
                             bias=eps_sb[:], scale=1.0)

        # ---- DMA stream (single SP queue; order == transfer order).
        # M[k] chunks pace PE's k-outer mm1; u halves interleave early so
        # the ACT Ln chain overlaps mm1 instead of trailing it ----
        M_sb = wp.tile([128, 8, 3 * BB], f16)
        u_sb = wp.tile([128, 4, 2 * A_DIM], mybir.dt.uint16)
        bb_sb = wp.tile([128, 10], f32)
        W2_sb = wp.tile([128, 8 * D_HEAD], f16)
        Wd_sb = wp.tile([128, 2 * A_DIM], f16)

        nc.sync.dma_start(M_sb[:, 0, 0:2 * BB], Mv[:, 0, 0:2 * BB])
        nc.sync.dma_start(M_sb[:, 0, 2 * BB:], Mv[:, 0, 2 * BB:])
        nc.sync.dma_start(M_sb[:, 1, :], Mv[:, 1, :])
        nc.sync.dma_start(M_sb[:, 2:4, :], Mv[:, 2:4, :])
        nc.sync.dma_start(u_sb[:, 0, :], u2v[:, 0, 0:2 * A_DIM])
        nc.sync.dma_start(M_sb[:, 4:6, :], Mv[:, 4:6, :])
        nc.sync.dma_start(u_sb[:, 1, :], u2v[:, 0, 2 * A_DIM:])
        nc.sync.dma_start(u_sb[:, 2, :], u2v[:, 1, 0:2 * A_DIM])
        nc.sync.dma_start(bb_sb[:], bbd[:])
        nc.sync.dma_start(M_sb[:, 6:8, :], Mv[:, 6:8, :])
        nc.sync.dma_start(u_sb[:, 3, :], u2v[:, 1, 2 * A_DIM:])
        nc.sync.dma_start(W2_sb[:], W2d[:])
        nc.sync.dma_start(Wd_sb[:], Wdd[:])
        id_sb = wp.tile([128, 128], f16)
        nc.sync.dma_start(id_sb[:], idd[:])
        if has_db:
            db_sb = wp.tile([1, A_DIM], f16)
            nc.sync.dma_start(db_sb[:], dbd[:])
            ones_sb = wp.tile([1, 128], f16)
            nc.vector.memset(ones_sb[:].bitcast(f32), 1.0)

        def W1k(k, j):
            off = j * 128 if j < 4 else 2 * BB + (j - 4) * 128
            return M_sb[:, k, ds(off, 128)]

        def sTk(k):
            return M_sb[:, k, ds(BB, BB)]

        b1_sb = bb_sb[:, 0:8]
        b2_sb = bb_sb[:, 8:10]
        W2_v = W2_sb.rearrange("p (j d) -> p j d", j=8)
        Wd_v = Wd_sb.rearrange("p (a m) -> p a m", a=2)

        # ---- gumbel per quarter: p = ln(u16*2^-16 + eps) and the
        # reference's q = ln(-p + eps) (== ln(-p) exactly in f32), both on
        # ACT; the pair subtract gdn = q0 - q1 runs on GPSIMD, which is
        # otherwise idle — DVE's tail queue stays free for relus/compares ----
        # bs0/bs2 keep gdn in f32 for DVE's is_ge compare; bs1/bs3 get the
        # NEGATED gdn in f16 — it is accumulated into d_ps by an identity
        # matmul so ACT's Sign (saturating u8 cast) yields the one-hot bit
        r_t = wp.tile([128, 4, 2 * A_DIM], f32)
        lnr_t = wp.tile([128, 4, A_DIM], f32)
        nlnr_t = wp.tile([128, 4, A_DIM], f16)
        for bs in range(4):
            p_t = p_pool.tile([128, 2 * A_DIM], f32, name="p_t")
            nc.scalar.activation(p_t[:], u_sb[:, bs, :], AFT.Ln,
                                 bias=eps_sb[:], scale=1.0 / 65536.0)
            q_t = r_t[:, bs, :]
            nc.scalar.activation(q_t, p_t[:], AFT.Ln,
                                 bias=eps_sb[:], scale=-1.0)
            if bs % 2 == 0:
                nc.gpsimd.tensor_tensor(lnr_t[:, bs, :],
                                        q_t[0:128, 0::2], q_t[0:128, 1::2],
                                        OP.subtract)
            else:
                nc.gpsimd.tensor_tensor(nlnr_t[:, bs, :],
                                        q_t[0:128, 1::2], q_t[0:128, 0::2],
                                        OP.subtract)

        # ---- mm1: single 8-wide k-outer; j6/j7 borrow the two a-bank
        # slots (freed by their relus before mm2's first accumulate).
        # k0 runs j0-3 first (they only need the first half-chunk M0a) ----
        h_pss = [(hd_psp if j < 6 else a_psp).tile(
            [128, BB], f32, name="h_ps",
            tag="ps" if j < 6 else "a") for j in range(8)]
        for k in range(8):
            jorder = (6, 7, 0, 1, 2, 3, 4, 5) if k == 7 else range(8)
            for j in jorder:
                nc.tensor.matmul(h_pss[j][:], W1k(k, j),
                                 sTk(k), start=(k == 0), stop=(k == 7))

        # ---- relus, split DVE/ACT so each lands before mm2 consumes it:
        # DVE starts at mm1's k7 column (j6 first), ACT joins once its Ln
        # chain drains ----
        hT_ts = [None] * 8
        for j in (6, 7, 0, 1, 2, 3, 4, 5):
            hT_t = hT_pool.tile([128, BB], f16, name="hT_t")
            if j in (6, 7, 1, 4):
                nc.vector.tensor_scalar(hT_t[:], h_pss[j][:],
                                        b1_sb[:, j:j + 1], 0.0,
                                        OP.add, OP.max)
            else:
                nc.scalar.activation(hT_t[:], h_pss[j][:], AFT.Relu,
                                     bias=b1_sb[:, j:j + 1], scale=1.0)
            hT_ts[j] = hT_t

        # ---- mm2 in relu-completion order, dt-interleaved except the
        # tail: j4/j5's dt0 partials run before their dt1 twins so
        # a_ps[0] stops two matmuls early — its awT then overlaps mm2's
        # tail and mm3's dt0 sweep starts the moment mm2 ends ----
        a_pss = [a_psp.tile([128, BB], f32, name="a_ps", tag="a")
                 for _ in range(2)]
        HB = BB // 2
        for j in (6, 7, 0, 1, 2):
            for dt_ in range(2):
                nc.tensor.matmul(
                    a_pss[dt_][:], W2_v[:, j, ts(dt_, 128)],
                    hT_ts[j][:], start=(j == 6), stop=False,
                    skip_group_check=True)
        for j, dt_ in ((4, 0), (3, 0), (5, 0), (3, 1), (4, 1), (5, 1)):
            nc.tensor.matmul(
                a_pss[dt_][:], W2_v[:, j, ts(dt_, 128)],
                hT_ts[j][:], start=False, stop=(j == 5),
                skip_group_check=True)

        # awT bias-adds split per half and across DVE/ACT
        awT_ts = [[None, None], [None, None]]
        for hf in range(2):
            for dt_ in range(2):
                awT_t = awT_pool.tile([128, HB], f16, name="awT_t")
                a_half = a_pss[dt_][:, ds(hf * HB, HB)]
                if dt_ == 0:
                    nc.vector.tensor_scalar_add(awT_t[:], a_half,
                                                b2_sb[:, dt_:dt_ + 1])
                else:
                    nc.scalar.activation(awT_t[:], a_half, AFT.Identity,
                                         bias=b2_sb[:, dt_:dt_ + 1],
                                         scale=1.0)
                awT_ts[dt_][hf] = awT_t

        # ---- mm3 + compare per 128-row group: all dt0 partials sweep
        # first (their awT is ready mid-mm2), then per-bs dt1(+identity)
        # stops feed the compares.  bs0/bs2 compare on DVE (is_ge vs lnr),
        # bs1/bs3 fold -gdn into PSUM via an identity matmul and emit on
        # ACT (Sign -> saturating u8 {0,1}), so the two engines drain the
        # compare chain in parallel ----
        o_t = wp.tile([128, 4, A_DIM], u8)
        d_pss = [hd_psp.tile([128, A_DIM], f32, name="d_ps", tag="ps")
                 for _ in range(4)]
        for bs in range(4):
            nc.tensor.matmul(
                d_pss[bs][:], awT_ts[0][bs // 2][:, ts(bs % 2, 128)],
                Wd_v[:, 0, :], start=True, stop=False)
        for bs in range(4):
            sgn = bs % 2 == 1
            d_ps = d_pss[bs]
            nc.tensor.matmul(
                d_ps[:], awT_ts[1][bs // 2][:, ts(bs % 2, 128)],
                Wd_v[:, 1, :], start=False,
                stop=not has_db and not sgn)
            if has_db:
                nc.tensor.matmul(d_ps[:], ones_sb[:], db_sb[:],
                                 start=False, stop=not sgn)
            if sgn:
                nc.tensor.matmul(d_ps[:], id_sb[:], nlnr_t[:, bs, :],
                                 start=False, stop=True)
                nc.scalar.activation(o_t[:, bs, :], d_ps[:], AFT.Sign,
                                     bias=eps_sb[:], scale=1.0)
            else:
                nc.vector.tensor_tensor(o_t[:, bs, :], d_ps[:],
                                        lnr_t[:, bs, :], OP.is_ge)
        # stores in dependency order: each store's data is ready by the
        # time the serial SEQ+HWDGE pipeline reaches it, so only the last
        # 128KB trails the final compare
        nc.sync.dma_start(outv[:, 0, 0:1, :], o_t[:, 0:1, :])
        nc.sync.dma_start(outv[:, 0, 1:2, :], o_t[:, 1:2, :])
        nc.sync.dma_start(outv[:, 0, 2:4, :], o_t[:, 2:4, :])

    nc.compile()
    return nc


def _build(rpc: int, has_db: bool):
    """Build the per-core kernel for `rpc` batch rows per core."""
    nb = rpc // BB
    nc = bacc.Bacc("TRN2", target_bir_lowering=False, debug=False,
                   num_devices=N_CORES)

    sTd = nc.dram_tensor("sT", [S_DIM, rpc], f16, kind="ExternalInput").ap()
    u2d = nc.dram_tensor("u2", [rpc, 2 * A_DIM], mybir.dt.uint16,
                         kind="ExternalInput").ap()
    W1d = nc.dram_tensor("W1d", [S_DIM, H_DIM], f16, kind="ExternalInput").ap()
    # W2/Wd arrive host-packed partition-major so one partition's data is a
    # single contiguous run (DMA chunk size drives HBM efficiency)
    W2d = nc.dram_tensor("W2d", [128, 8 * D_HEAD], f16, kind="ExternalInput").ap()
    Wdd = nc.dram_tensor("Wdd", [128, 2 * A_DIM], f16, kind="ExternalInput").ap()
    bbd = nc.dram_tensor("bbd", [128, 10], f32, kind="ExternalInput").ap()
    if has_db:
        dbd = nc.dram_tensor("dbd", [1, A_DIM], f16, kind="ExternalInput").ap()
    # only the even elements of each output pair are shipped (odd = 1 - even),
    # as uint8 {0,1} — exact, since the fp32 output is exactly one-hot
    outd = nc.dram_tensor("out", [rpc, A_DIM], u8, kind="ExternalOutput").ap()

    sTv = sTd.rearrange("(a p) b -> p a b", p=128)      # [128, 8, rpc]
    # u arrives host-permuted in row pairs: row = q*256 + p*2 + h, so each
    # partition line of a pair-load is 8KB contiguous
    u2v = u2d.rearrange("(q p h) m -> p q (h m)", p=128, h=2)
    # out leaves partition-major within each 2-block group: DRAM row
    # g*(128*x) + p*x + xx, so each group store writes x*512 contiguous
    # bytes per partition (host undoes the permutation)
    xgrp = 8 if nb % 2 == 0 else 4
    assert nb % 2 == 0 or nb == 1
    outv = outd.rearrange("(g p x) m -> p g x m", p=128, x=xgrp)

    with tile.TileContext(nc) as tc, ExitStack() as ctx:
        wp = ctx.enter_context(tc.tile_pool(name="weights", bufs=1))
        sT_pool = ctx.enter_context(tc.tile_pool(name="sTp", bufs=4))
        u_pool = ctx.enter_context(tc.tile_pool(name="up", bufs=3))
        p_pool = ctx.enter_context(tc.tile_pool(name="pp", bufs=1))
        lnr_pool = ctx.enter_context(tc.tile_pool(name="lnrp", bufs=3))
        gdn_pool = ctx.enter_context(tc.tile_pool(name="gdnp", bufs=2))
        hT_pool = ctx.enter_context(tc.tile_pool(name="hTp", bufs=10))
        awT_pool = ctx.enter_context(tc.tile_pool(name="awTp", bufs=6))
        out_pool = ctx.enter_context(tc.tile_pool(name="outp", bufs=3))
        # h and d tiles share one 6-slot tag so mm1 (block0's k-outer) and
        # mm3 time-share PSUM banks; awT holds the other 2 banks.
        hd_psp = ctx.enter_context(tc.tile_pool(name="hdps", bufs=6, space="PSUM"))
        a_psp = ctx.enter_context(tc.tile_pool(name="aps", bufs=2, space="PSUM"))

        W1v = W1d.rearrange("(a p) j -> p a j", p=128)

        ngrp = (nb + 1) // 2

        def g_cols(g):
            return min(2 * BB, rpc - g * 2 * BB)

        # ---- PE warmup: tiny bf16 matmuls so the p-state ramp runs while
        # the first real chunks are still in flight ----
        warm_sb = wp.tile([128, 128], bf16)
        nc.vector.memset(warm_sb[:].bitcast(f32), 0.0)
        warm_ps = hd_psp.tile([128, BB], f32, name="warm_ps", tag="ps")
        for _ in range(22):
            nc.tensor.matmul(warm_ps[:, 0:128], warm_sb[:], warm_sb[:],
                             start=True, stop=True)

        # ---- DMA stream (SP queue order == transfer order) ----
        bb_sb = wp.tile([128, 10], f32)
        nc.sync.dma_start(bb_sb[:], bbd[:])
        b1_sb = bb_sb[:, 0:8]
        b2_sb = bb_sb[:, 8:10]

        W1_sb = wp.tile([128, 8, H_DIM], f16)
        sT0_ts = []
        for ka in range(2):
            sT_t = sT_pool.tile([128, 4, 2 * BB], f16, name="sT_t")
            sT0_ts.append(sT_t)
        u_ts_all = [None] * nb

        def load_u(b0, uq=None):
            if uq is None or uq == 0:
                u_t = u_pool.tile([128, 4, 2 * A_DIM], mybir.dt.uint16,
                                  name="u_t")
                u_ts_all[b0] = u_t
            else:
                u_t = u_ts_all[b0]
            if uq is None:
                nc.sync.dma_start(
                    u_t[:].rearrange("p (q h) m -> p q (h m)", q=2),
                    u2v[:, 2 * b0:2 * b0 + 2, :])
            else:
                nc.sync.dma_start(
                    u_t[:, uq, :],
                    u2v[:, 2 * b0 + uq // 2,
                        ds((uq % 2) * 2 * A_DIM, 2 * A_DIM)])

        # block0 chunks: W1/sT stream fine-grained at the head (earliest PE
        # start), then 2-k-wide merged DMAs — the DMA dispatch pipeline
        # costs ~1.2us per DMA regardless of size, so small chunks waste
        # dispatch slots and push the u quarters (and the ACT Ln chain) late
        nc.sync.dma_start(W1_sb[:, 0, 0:256], W1v[:, 0, 0:256])
        nc.sync.dma_start(sT0_ts[0][:, 0, 0:g_cols(0)],
                          sTv[:, 0, ds(0, g_cols(0))])
        nc.sync.dma_start(W1_sb[:, 0, 256:1024], W1v[:, 0, 256:1024])
        nc.sync.dma_start(W1_sb[:, 1, :], W1v[:, 1, :])
        nc.sync.dma_start(sT0_ts[0][:, 1, 0:g_cols(0)],
                          sTv[:, 1, ds(0, g_cols(0))])
        if nb == 1:
            # merged 2-k chunks: at this size the stream is dispatch-bound
            # (~1.2us per DMA), not bandwidth-bound; the first sT pair stays
            # split so k2's column isn't gated on k3's bytes
            for k0_ in (2, 4, 6):
                nc.sync.dma_start(W1_sb[:, k0_:k0_ + 2, :],
                                  W1v[:, k0_:k0_ + 2, :])
                for kc in (k0_, k0_ + 1):
                    nc.sync.dma_start(sT0_ts[kc // 4][:, kc % 4, 0:g_cols(0)],
                                      sTv[:, kc, ds(0, g_cols(0))])
        else:
            # per-k chunks: 1MB sT slices here, so finer grain keeps mm1 fed
            for k in range(2, 8):
                nc.sync.dma_start(W1_sb[:, k, :], W1v[:, k, :])
                nc.sync.dma_start(sT0_ts[k // 4][:, k % 4, 0:g_cols(0)],
                                  sTv[:, k, ds(0, g_cols(0))])
        # nb==1: W2 late (mm2 is late, u's Ln chain is the critical tail);
        # nb>1: W2 before u (block0's mm2 comes sooner)
        W2_sb = wp.tile([128, 8 * D_HEAD], f16)
        W2_v = W2_sb.rearrange("p (j d) -> p j d", j=8)
        if nb > 1:
            nc.sync.dma_start(W2_sb[:], W2d[:])
        for uq in range(3):
            load_u(0, uq=uq)
        if nb == 1:
            nc.sync.dma_start(W2_sb[:], W2d[:])
        load_u(0, uq=3)
        Wd_sb = wp.tile([128, 2 * A_DIM], f16)
        nc.sync.dma_start(Wd_sb[:], Wdd[:])
        Wd_v = Wd_sb.rearrange("p (a m) -> p a m", a=2)
        eps_sb = wp.tile([128, 1], f32)
        nc.vector.memset(eps_sb[:], EPS)
        if has_db:
            db_sb = wp.tile([1, A_DIM], f16)
            nc.sync.dma_start(db_sb[:], dbd[:])
            ones_sb = wp.tile([1, 128], f16)
            nc.vector.memset(ones_sb[:].bitcast(f32), 1.0)
        if nb > 1:
            load_u(1)

        cur_sT = sT0_ts
        next_sT = None
        o_t = None

        def sT_group_load(g):
            tiles = []
            for ka in range(2):
                sT_t = sT_pool.tile([128, 4, 2 * BB], f16, name="sT_t")
                nc.sync.dma_start(
                    sT_t[:, :, 0:g_cols(g)],
                    sTv[:, ka * 4:(ka + 1) * 4, ds(g * 2 * BB, g_cols(g))])
                tiles.append(sT_t)
            return tiles

        for b0 in range(nb):
            g = b0 // 2
            bw = b0 % 2
            if bw == 0:
                o_t = out_pool.tile([128, 8, A_DIM], u8, name="o_t")
            else:
                # prefetch the NEXT 2-block sT group one block ahead — late
                # enough to keep the head DMA queue short, early enough that
                # the 2MB lands within one block period
                if g + 1 < ngrp:
                    next_sT = sT_group_load(g + 1)
            if b0 + 2 < nb:
                load_u(b0 + 2)

            u_t = u_ts_all[b0]

            def sT_at(k):
                return cur_sT[k // 4][:, k % 4, ds(bw * BB, BB)]

            # ---- gumbel, ratio-domain.  p = ln(u+eps) (< 0 always), and the
            # reference's q = ln(-p+eps) satisfies -p+eps == -p exactly in
            # f32 (|p| >= 6e-8 >> 1e-20), so
            #   gdn = q0-q1 = ln(E0/E1) = ln(p0/p1) = ln(R).
            # One Ln pass + a strided divide (Pool) + a half-size Ln replace
            # Ln+Ln+subtract; ACT only ever runs Ln/Relu/Identity, which
            # share one activation table (no LoadActFuncSet churn). ----
            r_t = gdn_pool.tile([128, 4, 2 * A_DIM], f32, name="r_t")
            lnr_t = lnr_pool.tile([128, 4, A_DIM], f32, name="lnr_t")

            def q_quarter(bs):
                # block0: per-bs gumbel so each quarter's chain starts the
                # moment its u DMA lands
                p_t = p_pool.tile([128, 2 * A_DIM], f32, name="p_t")
                nc.scalar.activation(p_t[:], u_t[:, bs, :], AFT.Ln,
                                     bias=eps_sb[:], scale=1.0 / 65536.0)
                q_t = r_t[:, bs, :]
                nc.scalar.activation(q_t, p_t[:], AFT.Ln,
                                     bias=eps_sb[:], scale=-1.0)

            def gdn_quarter(bs):
                q_t = r_t[:, bs, :]
                nc.vector.tensor_tensor(lnr_t[:, bs, :],
                                        q_t[0:128, 0::2], q_t[0:128, 1::2],
                                        OP.subtract)

            if b0 == 0:
                for bs in range(4):
                    q_quarter(bs)
                gdn_quarter(0)
                gdn_quarter(1)
            else:
                p_t = p_pool.tile([128, 4, 2 * A_DIM], f32, name="p_t")
                nc.scalar.activation(
                    p_t[:].rearrange("p h m -> p (h m)"),
                    u_t[:].rearrange("p h m -> p (h m)"),
                    AFT.Ln, bias=eps_sb[:], scale=1.0 / 65536.0)
                nc.scalar.activation(
                    r_t[:].rearrange("p q m -> p (q m)"),
                    p_t[:].rearrange("p h m -> p (h m)"),
                    AFT.Ln, bias=eps_sb[:], scale=-1.0)

            # ---- mm1 (+ mm2 interleaved one j-group behind for b0>0) ----
            hT_ts = [None] * 8

            def relu(j, h_ps, eng):
                hT_t = hT_pool.tile([128, BB], f16, name="hT_t")
                if eng == "act":
                    nc.scalar.activation(hT_t[:], h_ps[:], AFT.Relu,
                                         bias=b1_sb[:, j:j + 1], scale=1.0)
                else:
                    nc.vector.tensor_scalar(hT_t[:], h_ps[:],
                                            b1_sb[:, j:j + 1], 0.0,
                                            OP.add, OP.max)
                hT_ts[j] = hT_t

            def mm2_partial(j, j_start=0, j_stop=7):
                for dt_ in range(2):
                    nc.tensor.matmul(a_pss[dt_][:], W2_v[:, j, ts(dt_, 128)],
                                     hT_ts[j][:], start=(j == j_start),
                                     stop=(j == j_stop), skip_group_check=True)

            if b0 == 0:
                # single 8-wide k-outer: j6/j7 borrow the two a-bank slots
                # (freed by their relus before mm2's first accumulate), so PE
                # streams mm1 with no group barrier.  k7's column runs j6/j7
                # first so the relu pipeline frees those banks earliest.
                h_pss = [(hd_psp if j < 6 else a_psp).tile(
                    [128, BB], f32, name="h_ps",
                    tag="ps" if j < 6 else "a") for j in range(8)]
                for k in range(8):
                    jorder = (6, 7, 0, 1, 2, 3, 4, 5) if k == 7 else range(8)
                    for j in jorder:
                        nc.tensor.matmul(
                            h_pss[j][:], W1_sb[:, k, ts(j, 128)],
                            sT_at(k), start=(k == 0), stop=(k == 7))
                # relus: DVE leads (ACT drains the u Lns first, then takes
                # the tail); emitted j6/j7-first to free the borrowed a-banks
                for i, j in enumerate((6, 7, 0, 1, 2, 3, 4, 5)):
                    relu(j, h_pss[j], "dve" if i < 6 else "act")
                a_pss = [a_psp.tile([128, BB], f32, name="a_ps", tag="a")
                         for _ in range(2)]
                # accumulate mm2 in relu-completion order so its first
                # matmuls fire as soon as relu j6 lands
                for j in (6, 7, 0, 1, 2, 3, 4, 5):
                    mm2_partial(j, j_start=6, j_stop=5)
            else:
                a_pss = [a_psp.tile([128, BB], f32, name="a_ps", tag="a")
                         for _ in range(2)]
                # last block: run j6/j7 first so their DVE relus finish
                # during mm1 and the tail's mm2 partials wait only on ACT
                # relus that are already done by mm1's end
                last = b0 == nb - 1
                jorder = (6, 7, 0, 1, 2, 3, 4, 5) if last else range(8)
                js, je = (6, 5) if last else (0, 7)
                for i, j in enumerate(jorder):
                    # the first j borrows an a-bank slot (free across the
                    # block boundary) so this block's first mm1 writes never
                    # wait on the previous block's d_ps compares
                    h_ps = (a_psp if i == 0 else hd_psp).tile(
                        [128, BB], f32, name="h_ps",
                        tag="a" if i == 0 else "ps")
                    for k in range(8):
                        nc.tensor.matmul(h_ps[:], W1_sb[:, k, ts(j, 128)],
                                         sT_at(k), start=(k == 0),
                                         stop=(k == 7))
                    # j6/j7 relus on DVE so the whole-block lnR Ln (queued
                    # on ACT after relu j5) never delays mm2's tail
                    relu(j, h_ps, "act" if j < 6 else "dve")
                    if i == 5:
                        nc.vector.tensor_tensor(
                            lnr_t[:], r_t[:, :, 0::2], r_t[:, :, 1::2],
                            OP.subtract)
                    if i >= 2:
                        mm2_partial(jorder[i - 2] if last else j - 2,
                                    j_start=js, j_stop=je)
                mm2_partial(jorder[6] if last else 6, j_start=js, j_stop=je)
                mm2_partial(jorder[7] if last else 7, j_start=js, j_stop=je)

            # awT split per half and across DVE/ACT so mm3's first matmul
            # waits only ~0.4us of bias-add, not the full-width pair
            awT_ts = [[None, None], [None, None]]
            for hf in range(2):
                for dt_ in range(2):
                    awT_t = awT_pool.tile([128, BB // 2], f16, name="awT_t")
                    a_half = a_pss[dt_][:, ds(hf * (BB // 2), BB // 2)]
                    if dt_ == 0:
                        nc.vector.tensor_scalar_add(awT_t[:], a_half,
                                                    b2_sb[:, dt_:dt_ + 1])
                    else:
                        nc.scalar.activation(awT_t[:], a_half, AFT.Identity,
                                             bias=b2_sb[:, dt_:dt_ + 1],
                                             scale=1.0)
                    awT_ts[dt_][hf] = awT_t
            if b0 == 0:
                gdn_quarter(2)
                gdn_quarter(3)

            # ---- mm3 + compare/emit per 128-row group (d_ps lives in
            # PSUM, so the compares must run on DVE) ----
            for bs in range(4):
                d_ps = hd_psp.tile([128, A_DIM], f32, name="d_ps", tag="ps")
                for dt_ in range(2):
                    nc.tensor.matmul(
                        d_ps[:], awT_ts[dt_][bs // 2][:, ts(bs % 2, 128)],
                        Wd_v[:, dt_, :], start=(dt_ == 0),
                        stop=(dt_ == 1 and not has_db))
                if has_db:
                    nc.tensor.matmul(d_ps[:], ones_sb[:], db_sb[:],
                                     start=False, stop=True)
                nc.vector.tensor_tensor(o_t[:, bw * 4 + bs, :], d_ps[:],
                                        lnr_t[:, bs, :], OP.is_ge)
            last_of_group = (bw == 1) or (b0 == nb - 1)
            if last_of_group:
                nx = 4 * (bw + 1)
                if b0 == nb - 1:
                    # split the tail store so only 64KB trails the last
                    # compare
                    nc.sync.dma_start(outv[:, g, 0:nx - 2, :],
                                      o_t[:, 0:nx - 2, :])
                    nc.sync.dma_start(outv[:, g, nx - 2:nx - 1, :],
                                      o_t[:, nx - 2:nx - 1, :])
                    nc.sync.dma_start(outv[:, g, nx - 1:nx, :],
                                      o_t[:, nx - 1:nx, :])
                else:
                    nc.sync.dma_start(outv[:, g, 0:nx, :], o_t[:, 0:nx, :])
                cur_sT = next_sT

    nc.compile()
    return nc


def kernel(s, u, W1, b1, W2, b2, head_W, head_b, _rpc=None):
    global LAST_EXEC_NS
    s = np.asarray(s, dtype=np.float32)
    u = np.asarray(u, dtype=np.float32)
    W1 = np.asarray(W1, dtype=np.float32)
    W2 = np.asarray(W2, dtype=np.float32)
    b1 = np.asarray(b1, dtype=np.float32)
    b2 = np.asarray(b2, dtype=np.float32)
    head_W = np.asarray(head_W, dtype=np.float32)
    head_b = np.asarray(head_b, dtype=np.float32)

    nrows = s.shape[0]
    rpc = _rpc if _rpc is not None else nrows // N_CORES
    assert nrows == rpc * N_CORES and rpc % BB == 0

    sT = np.ascontiguousarray(s.T.astype(np.float16))   # [S_DIM, nrows] f16
    # u as uint16 fixed-point: round(u * 2^16), clamped.  Absolute error
    # 2^-17 beats f16's relative error where the gumbel transform is
    # sensitive (u near 1) at the same 2 bytes/elem.
    u2 = np.minimum(np.round(u.reshape(nrows, 2 * A_DIM) * 65536.0),
                    65535.0).astype(np.uint16)
    Wd = np.ascontiguousarray((head_W[:, :, 0] - head_W[:, :, 1]).T)
    W1h = W1.astype(np.float16)
    W2h = np.ascontiguousarray(
        W2.astype(np.float16).reshape(8, 128, D_HEAD).transpose(1, 0, 2)
    ).reshape(128, 8 * D_HEAD)
    Wdh = np.ascontiguousarray(
        Wd.astype(np.float16).reshape(2, 128, A_DIM).transpose(1, 0, 2)
    ).reshape(128, 2 * A_DIM)
    db = np.ascontiguousarray(head_b[:, 0] - head_b[:, 1]).reshape(1, A_DIM)
    has_db = bool(np.any(db))
    bb = np.concatenate([b1.reshape(8, 128).T, b2.reshape(2, 128).T], axis=1)
    bb = np.ascontiguousarray(bb, dtype=np.float32)     # [128, 10]

    key = (rpc, has_db)
    if key not in _cache:
        _cache[key] = (_build_small(rpc, has_db) if rpc == BB
                       else _build(rpc, has_db))
    nc = _cache[key]

    nq = rpc // 256
    in_maps = []
    for c in range(N_CORES):
        uc = u2[c * rpc:(c + 1) * rpc]
        up = np.ascontiguousarray(
            uc.reshape(nq, 2, 128, 2 * A_DIM).transpose(0, 2, 1, 3)
        ).reshape(rpc, 2 * A_DIM)
        m = {
            "u2": up,
            "W2d": W2h, "Wdd": Wdh, "bbd": bb,
        }
        if rpc == BB:
            # small path: W1 and sT interleaved per k chunk as
            # [W1 j0-3 | sT | W1 j4-7]
            m["Md"] = np.ascontiguousarray(np.concatenate(
                [W1h[:, 0:512], sT[:, c * rpc:(c + 1) * rpc],
                 W1h[:, 512:1024]], axis=1))
            m["idd"] = np.eye(128, dtype=np.float16)
        else:
            m["sT"] = np.ascontiguousarray(sT[:, c * rpc:(c + 1) * rpc])
            m["W1d"] = W1h
        if has_db:
            m["dbd"] = db.astype(np.float16)
        in_maps.append(m)

    res = bass_utils.run_bass_kernel_spmd(nc, in_maps,
                                          core_ids=list(range(N_CORES)))
    LAST_EXEC_NS = res.exec_time_ns
    nb = rpc // BB
    xgrp = 8 if nb % 2 == 0 else 4
    shards = []
    for c in range(N_CORES):
        e = res.results[c]["out"]                        # [rpc, A_DIM] uint8
        # undo the (g, p, x) store permutation back to batch order
        e = e.reshape(rpc // (128 * xgrp), 128, xgrp, A_DIM)
        shards.append(e.transpose(0, 2, 1, 3).reshape(rpc, A_DIM))
    evens = np.concatenate(shards, axis=0)               # [nrows, A_DIM]
    out = np.empty((nrows, 2 * A_DIM), dtype=np.float32)
    ef = evens.astype(np.float32)
    out[:, 0::2] = ef
    out[:, 1::2] = 1.0 - ef
    return out

